# revision 1
# baseline (speedup 1.0000x reference)
"""Multi-head attention (B=2, S=2048, D=1024, H=16, no mask) on 8 TRN2 cores.

Sharding: tensor-parallel over heads — 2 heads per core. Each core computes
its heads' QKV projections, attention, and a partial out-projection
(row-sharded Wo); the host sums the 8 partials and adds the bias (the
all-reduce happens at gather time).

Device layout (per core):
  - All activations kept transposed: feat-on-partitions, tokens-on-free.
  - scoresT[k, q] = k @ qT (contract over HD via K=64 matmuls), exp on ACT
    (no max-subtraction needed: |scores| < ~3 by construction), PV via
    lhsT = [v | 1] (M=65) giving unnormalized ctxT plus a row of softmax
    denominators in the same matmul stream.
  - normalize: one DVE evacuation frees the ctx PSUM bank early; 1/denom
    via DVE reciprocal on a 16-partition scatter of the denom row;
    partition-broadcast of 1/denom via a K=1 PE matmul; fp32 multiply.
  - fp32r matmuls everywhere (full PE rate at N=512, ~2e-4 rel err);
    exp output and ctxT are fp32r so they can feed PE directly.
"""
import numpy as np

B = 2
S = 2048
D = 1024
H = 16
HD = 64
NCORES = 8
HPC = H // NCORES       # heads per core
FPC = HPC * HD          # 128 features per core


def build_mha_kernel(tc, outT, xT, wqT, wkT, wvT, woT, s=S, d=D):
    """Emit the per-core MHA program.

    outT: [B, d, s] f32 (partial output, transposed, per-batch)
    xT:   [B, d, s] f32r
    wqT/wkT/wvT: [128, d//128, FPC] f32r, host pre-arranged so the
        weight DMA is contiguous (wqT pre-scaled by 1/sqrt(HD))
    woT:  [FPC, d] f32r
    """
    import concourse.mybir as mybir
    from concourse.masks import make_identity
    from contextlib import ExitStack

    nc = tc.nc
    f32 = mybir.dt.float32
    f32r = mybir.dt.float32r
    Exp = mybir.ActivationFunctionType.Exp
    Ln = mybir.ActivationFunctionType.Ln

    KT = d // 128           # contraction tiles for projections
    SK = s // 128           # key tiles
    SQB = min(1024, s)      # query block (psum-resident ctx width)
    NBLK = s // SQB
    J = min(512, SQB)       # matmul free-dim
    NJ = SQB // J
    NCH = s // 512          # 512-token chunks

    with ExitStack() as es:
        consts = es.enter_context(tc.tile_pool(name="consts", bufs=1))
        wpool = es.enter_context(tc.tile_pool(name="w", bufs=1))
        xpool = es.enter_context(tc.tile_pool(name="xt", bufs=1))
        qkv = es.enter_context(tc.tile_pool(name="qkv", bufs=1))
        vapool = es.enter_context(tc.tile_pool(name="va", bufs=1))
        epool = es.enter_context(tc.tile_pool(name="exp", bufs=2))
        cpool = es.enter_context(tc.tile_pool(name="ctxT", bufs=2))
        spool = es.enter_context(tc.tile_pool(name="small", bufs=2))
        opool = es.enter_context(tc.tile_pool(name="o", bufs=4))
        ps_ctx = es.enter_context(tc.tile_pool(name="psctx", bufs=1, space="PSUM"))
        ps_sc = es.enter_context(tc.tile_pool(name="pssc", bufs=2, space="PSUM"))
        ps_wk = es.enter_context(tc.tile_pool(name="pswk", bufs=2, space="PSUM"))

        identity = consts.tile([128, 128], f32, tag="ident")
        make_identity(nc, identity[:])
        ones_f = consts.tile([128, 1], f32, tag="ones_f")
        nc.gpsimd.memset(ones_f[:], 1.0)
        ones_r = consts.tile([128, 1], f32r, tag="ones_r")
        nc.vector.tensor_copy(ones_r[:], ones_f[:])
        ones_f64 = consts.tile([1, HD], f32, tag="ones_f64")
        nc.gpsimd.memset(ones_f64[:], 1.0)
        ones1x64 = consts.tile([1, HD], f32r, tag="ones1x64")
        nc.vector.tensor_copy(ones1x64[:], ones_f64[:])

        # --- weights (resident for the whole kernel), host pre-arranged so
        # each DMA is one contiguous read per partition row. wq issues first
        # (first projection needs it); wk/wv/wo go on the ACT HWDGE queue,
        # idle at startup, so they don't delay the batch-0 x loads on SP.
        wq_sb = wpool.tile([128, KT, FPC], f32r, tag="wq")
        nc.sync.dma_start(wq_sb[:], wqT)
        wk_sb = wpool.tile([128, KT, FPC], f32r, tag="wk")
        nc.scalar.dma_start(wk_sb[:], wkT)
        wv_sb = wpool.tile([128, KT, FPC], f32r, tag="wv")
        nc.scalar.dma_start(wv_sb[:], wvT)
        wo_sb = wpool.tile([128, d], f32r, tag="wo")
        nc.scalar.dma_start(wo_sb[:], woT)

        def emit_outproj(ctxT_b, bb, m, ch):
            # one partial out-projection unit: outT[bb] tile (m, ch)
            ms = slice(m * 128, (m + 1) * 128)
            cs = slice(ch * 512, (ch + 1) * 512)
            op = ps_wk.tile([128, 512], f32, tag="wk")
            nc.tensor.matmul(op[:], wo_sb[:, ms], ctxT_b[:, cs],
                             start=True, stop=True)
            ot = opool.tile([128, 512], f32, tag="ot")
            if bb == B - 1:
                nc.scalar.copy(ot[:], op[:])
            else:
                nc.vector.tensor_copy(ot[:], op[:])
            nc.sync.dma_start(outT[bb, ms, cs], ot[:])

        # out-projection units of batch b are emitted interleaved into batch
        # b+1's attention sk-loop so they fill the ACT-paced PE bubbles
        pending = []

        for b in range(B):
            # --- load xT[b]; batch 0 splits across the SP and ACT HWDGE
            # queues (ACT is idle before the first exp) to halve startup
            xts = []
            for k in range(KT):
                xt = xpool.tile([128, s], f32r, tag=f"x{k}")
                eng = nc.scalar if (b == 0 and k % 2 == 1) else nc.sync
                eng.dma_start(xt[:], xT[b, k * 128:(k + 1) * 128, :])
                xts.append(xt)

            # --- projections: pT = W_l @ xT  -> [feat(128), tokens(s)]
            qT = qkv.tile([128, s], f32r, tag="q")
            kT = qkv.tile([128, s], f32r, tag="k")
            vT = qkv.tile([128, s], f32, tag="v")
            for w_sb, dst in ((wq_sb, qT), (wk_sb, kT), (wv_sb, vT)):
                for n in range(NCH):
                    pt = ps_wk.tile([128, 512], f32, tag="wk")
                    for k in range(KT):
                        nc.tensor.matmul(
                            pt[:], w_sb[:, k, :], xts[k][:, n * 512:(n + 1) * 512],
                            start=(k == 0), stop=(k == KT - 1))
                    nc.vector.tensor_copy(dst[:, n * 512:(n + 1) * 512], pt[:])

            # --- v transpose + ones-augmented v tiles [tokens(128), HD+1]
            vas = {}
            for sk in range(SK):
                tp = ps_wk.tile([128, 128], f32, tag="wk")
                nc.tensor.transpose(tp[:], vT[:, sk * 128:(sk + 1) * 128], identity[:])
                for h in range(HPC):
                    va = vapool.tile([128, HD + 1], f32r, tag=f"va{h}_{sk}")
                    nc.vector.tensor_copy(va[:, 0:HD], tp[:, h * HD:(h + 1) * HD])
                    nc.vector.tensor_copy(va[:, HD:HD + 1], ones_r[:])
                    vas[(h, sk)] = va

            # --- attention, both heads paired per q-block of 512 columns:
            # the two K=64 QK matmuls alternate array row-groups 0/64, which
            # measures ~2-3x faster per pair than consecutive same-row-group
            # K=64 matmuls (weight-load conflict; see probe_pack.py). One exp
            # covers both heads' scores.
            ctxT = cpool.tile([128, s], f32r, tag="ctxT")
            NQB = s // 512
            for qb in range(NQB):
                qs = slice(qb * 512, (qb + 1) * 512)
                cpss = []
                for h in range(HPC):
                    cph = ps_ctx.tile([HD + 1, 512], f32, tag=f"ctx{h}")
                    cpss.append(cph)
                for sk in range(SK):
                    if pending:
                        pending.pop(0)()
                    sps = ps_sc.tile([128, 2 * 512], f32, tag="sc")
                    for h in range(HPC):
                        hr = slice(h * HD, (h + 1) * HD)
                        nc.tensor.matmul(
                            sps[:, h * 512:(h + 1) * 512],
                            kT[hr, sk * 128:(sk + 1) * 128],
                            qT[hr, qs], start=True, stop=True)
                    et = epool.tile([128, 2 * 512], f32r, tag="exp")
                    nc.scalar.activation(et[:], sps[:], Exp)
                    for h in range(HPC):
                        nc.tensor.matmul(
                            cpss[h][:], vas[(h, sk)][:],
                            et[:, h * 512:(h + 1) * 512],
                            start=(sk == 0), stop=(sk == SK - 1))
                    # Evacuate ctx+denom from psum in one copy (frees the ctx
                    # slot for the next block), then normalize off-psum:
                # 1/denom via DVE reciprocal on a 16-partition scatter of the
                # denom row (8 cyc/elem on one lane would be too slow),
                # broadcast across partitions via a K=1 matmul.
                for h in range(HPC):
                    cu = spool.tile([HD + 1, 512], f32, tag=f"cu{h}")
                    nc.vector.tensor_copy(cu[:], cpss[h][:, :])
                    d16 = spool.tile([16, 512 // 16], f32, tag=f"d16{h}")
                    nc.gpsimd.dma_start(d16[:], cu[HD:HD + 1, :])
                    r16 = spool.tile([16, 512 // 16], f32r, tag=f"r16{h}")
                    with nc.allow_low_precision(reason="1/denom rounded to f32r"):
                        nc.vector.reciprocal(r16[:], d16[:])
                    rd = spool.tile([1, 512], f32r, tag=f"rd{h}")
                    nc.gpsimd.dma_start(rd[:], r16[:])

                    def norm_tail(h=h, qs=qs, cu=cu, rd=rd, ctxT=ctxT):
                        bc = ps_wk.tile([HD, 512], f32, tag="wk")
                        nc.tensor.matmul(bc[:], ones1x64[:], rd[:],
                                         start=True, stop=True)
                        if h == 0:
                            nc.vector.tensor_mul(ctxT[0:HD, qs],
                                                 cu[0:HD, :], bc[:])
                        else:
                            cn = spool.tile([HD, 512], f32r, tag="cn")
                            nc.vector.tensor_mul(cn[:], cu[0:HD, :], bc[:])
                            nc.gpsimd.dma_start(ctxT[HD:2 * HD, qs], cn[:])

                    if b < B - 1 and qb == NQB - 1:
                        # defer the final normalize tails: their broadcast
                        # matmuls would sit in the PE FIFO waiting on the
                        # reciprocal chains and block the next batch
                        pending.append(norm_tail)
                    else:
                        norm_tail()
                if b == B - 1:
                    # last batch: this q-block's ctxT columns are final now;
                    # queue their out-proj units so the next block's
                    # attention absorbs them
                    pending.extend(
                        (lambda c=ctxT, bb=b, mm=m, cc=qb:
                         emit_outproj(c, bb, mm, cc))
                        for m in range(KT))

            # --- partial out projection: outT[b] = woT.T @ ctxT.
            # Queue this batch's units for the next batch's attention loop
            # (after any deferred normalize tail); the last batch emits
            # leftovers directly.
            if b == B - 1:
                for u in pending:
                    u()
                pending = []
            else:
                pending = pending + [
                    (lambda c=ctxT, bb=b, mm=m, cc=ch:
                     emit_outproj(c, bb, mm, cc))
                    for m in range(KT) for ch in range(NCH)]


_CACHE = {}


def _get_compiled(s=S, d=D, reps=1):
    key = (s, d, reps)
    if key not in _CACHE:
        import concourse.bacc as bacc
        import concourse.tile as tile
        import concourse.mybir as mybir

        f32 = mybir.dt.float32
        f32r = mybir.dt.float32r
        nc = bacc.Bacc("TRN2", target_bir_lowering=False, debug=False)
        xT = nc.dram_tensor("xT", [B, d, s], f32r, kind="ExternalInput")
        wqT = nc.dram_tensor("wqT", [128, d // 128, FPC], f32r,
                             kind="ExternalInput")
        wkT = nc.dram_tensor("wkT", [128, d // 128, FPC], f32r,
                             kind="ExternalInput")
        wvT = nc.dram_tensor("wvT", [128, d // 128, FPC], f32r,
                             kind="ExternalInput")
        woT = nc.dram_tensor("woT", [FPC, d], f32r, kind="ExternalInput")
        outT = nc.dram_tensor("outT", [B, d, s], f32, kind="ExternalOutput")
        with tile.TileContext(nc) as tc:
            for _ in range(reps):
                build_mha_kernel(tc, outT.ap(), xT.ap(), wqT.ap(), wkT.ap(),
                                 wvT.ap(), woT.ap(), s=s, d=d)
        nc.compile()
        _CACHE[key] = nc
    return _CACHE[key]


def make_in_maps(x, Wq, Wk, Wv, Wo):
    """Host-side shard prep: transpose x, slice + transpose weights per core."""
    b, s, d = x.shape
    xT = np.ascontiguousarray(x.transpose(0, 2, 1)).astype(np.float32)
    scale = np.float32(1.0 / np.sqrt(HD))

    def prearr(wt):
        # [d, FPC] -> [128, d//128, FPC] so each SBUF partition row is one
        # contiguous DMA line (avoids 512B-descriptor strided reads)
        return np.ascontiguousarray(
            wt.reshape(d // 128, 128, FPC).transpose(1, 0, 2)).astype(np.float32)

    in_maps = []
    for c in range(NCORES):
        if (c + 1) * FPC > d:
            # small-D sim configs: fewer head-slices than cores
            in_maps.append(in_maps[0])
            continue
        rs = slice(c * FPC, (c + 1) * FPC)
        in_maps.append({
            "xT": xT,
            "wqT": prearr((Wq[rs, :] * scale).T.astype(np.float32)),
            "wkT": prearr(Wk[rs, :].T.astype(np.float32)),
            "wvT": prearr(Wv[rs, :].T.astype(np.float32)),
            "woT": np.ascontiguousarray(Wo[:, rs].T).astype(np.float32),
        })
    return in_maps


_RUNNER = None
_RUNNER_STATE = {}


def _get_runner():
    """Build (once) a cached jitted SPMD executor mirroring
    bass2jax.run_bass_via_pjrt's multi-core path."""
    global _RUNNER
    if _RUNNER is None:
        import jax
        import jax.numpy as jnp
        from jax.sharding import Mesh, PartitionSpec
        from jax.experimental.shard_map import shard_map
        import concourse.mybir as mybir
        from concourse import bass2jax

        nc = _get_compiled()
        bass2jax.install_neuronx_cc_hook()

        partition_name = (nc.partition_id_tensor.name
                          if nc.partition_id_tensor else None)
        in_names = []
        out_names = []
        out_avals = []
        for alloc in nc.m.functions[0].allocations:
            if not isinstance(alloc, mybir.MemoryLocationSet):
                continue
            name = alloc.memorylocations[0].name
            if alloc.kind == "ExternalInput":
                if name != partition_name:
                    in_names.append(name)
            elif alloc.kind == "ExternalOutput":
                out_names.append(name)
                out_avals.append(jax.core.ShapedArray(
                    tuple(alloc.tensor_shape), mybir.dt.np(alloc.dtype)))
        n_params = len(in_names)
        n_outs = len(out_names)
        all_names = in_names + out_names
        if partition_name is not None:
            all_names = all_names + [partition_name]

        def _body(*args):
            operands = list(args)
            if partition_name is not None:
                operands.append(bass2jax.partition_id_tensor())
            outs = bass2jax._bass_exec_p.bind(
                *operands,
                out_avals=tuple(out_avals),
                in_names=tuple(all_names),
                out_names=tuple(out_names),
                lowering_input_output_aliases=(),
                sim_require_finite=True,
                sim_require_nnan=True,
                nc=nc,
            )
            return tuple(outs)

        devices = jax.devices()[:NCORES]
        mesh = Mesh(np.asarray(devices), ("core",))
        # xT is identical on every core: replicate it instead of concatenating
        # 8 copies on the host.
        in_specs = tuple(PartitionSpec() if name == "xT" else PartitionSpec("core")
                         for name in in_names)
        sharded = jax.jit(
            shard_map(_body, mesh=mesh,
                      in_specs=in_specs + (PartitionSpec("core"),) * n_outs,
                      out_specs=(PartitionSpec("core"),) * n_outs,
                      check_rep=False),
            keep_unused=True)

        # separate jit: on-device sum of the 8 per-core partials (all-reduce)
        def _reduce(a):
            return jnp.sum(a.reshape((NCORES,) + tuple(out_avals[0].shape)),
                           axis=0)
        reduce_jit = jax.jit(_reduce)

        out_shapes = [tuple(a.shape) for a in out_avals]
        out_dtypes = [a.dtype for a in out_avals]
        zeros_dev = [None]

        from jax.sharding import NamedSharding
        rep_shd = NamedSharding(mesh, PartitionSpec())

        def call(in_maps):
            args = []
            for name in in_names:
                if name == "xT":
                    # one host->device transfer, then device-side broadcast
                    xd = jax.device_put(np.asarray(in_maps[0][name]),
                                        devices[0])
                    args.append(jax.device_put(xd, rep_shd))
                else:
                    args.append(np.concatenate(
                        [np.asarray(m[name]) for m in in_maps], axis=0))
            if zeros_dev[0] is None:
                from jax.sharding import NamedSharding
                shd = NamedSharding(mesh, PartitionSpec("core"))
                zeros_dev[0] = [
                    jax.device_put(
                        np.zeros((NCORES * sh[0],) + sh[1:], dt), shd)
                    for sh, dt in zip(out_shapes, out_dtypes)]
            outs = sharded(*args, *zeros_dev[0])
            try:
                summed = np.asarray(reduce_jit(outs[0]))
            except Exception:
                # device reduce unavailable: fetch partials, sum on host
                a = np.asarray(outs[0])
                summed = a.reshape((NCORES,) + tuple(out_avals[0].shape)).sum(0)
            return {out_names[0]: summed}

        _RUNNER_STATE.update(sharded=sharded, in_names=in_names,
                             out_shapes=out_shapes, out_dtypes=out_dtypes,
                             call=call, mesh=mesh)
        _RUNNER = call
    return _RUNNER


def run(x, Wq, Wk, Wv, Wo, bo, trace=False):
    from concourse._compat import axon_active
    in_maps = make_in_maps(x, Wq, Wk, Wv, Wo)
    if axon_active():
        summed = _get_runner()(in_maps)
        acc = summed["outT"].astype(np.float64)
        results = summed
    else:
        # native /dev/neuron* path (non-axon environments)
        from concourse import bass_utils
        r = bass_utils.run_bass_kernel_spmd(
            _get_compiled(), in_maps, core_ids=list(range(NCORES)), trace=trace)
        results = r.results
        acc = np.zeros((B, D, S), dtype=np.float64)
        for c in range(NCORES):
            acc += results[c]["outT"]
    out = acc.transpose(0, 2, 1) + np.asarray(bo, dtype=np.float64)
    return out.astype(np.float32), results


def kernel(x, Wq, Wk, Wv, Wo, bo):
    out, _ = run(np.asarray(x), np.asarray(Wq), np.asarray(Wk),
                 np.asarray(Wv), np.asarray(Wo), np.asarray(bo))
    return out



# revision 34
# speedup vs baseline: 1.3141x; 1.3141x over previous
"""Multi-head attention (B=2, S=2048, D=1024, H=16, no mask) on 8 TRN2 cores.

Sharding: tensor-parallel over heads — 2 heads per core. Each core computes
its heads' QKV projections, attention, and a partial out-projection
(row-sharded Wo); the host sums the 8 partials and adds the bias (the
all-reduce happens at gather time).

Device layout (per core), v3 — bf16 dataflow, software-pipelined attention,
transposed PV with per-partition softmax normalization:
  - All tensor data bf16 (x, W, q/k/v, exp-scores, ctx, partial out);
    matmul accumulation and softmax statistics stay fp32 in PSUM.
  - qT/kT kept transposed (feat-on-partitions); v projected DIRECTLY in
    token-major layout ([tokens, head, HD+1] va tiles, ones column
    prebaked) via per-token-tile matmuls — no PE transpose pass for v.
  - scoresT[k, q] = k @ qT per (head, key-tile): the two heads' K=64
    matmuls sit on array row-groups 0/64 and run concurrently on HW.
  - The attention inner loop is software-pipelined: QK(sk+1) is emitted
    BEFORE PV(sk), so the in-order PE never parks the next score matmul
    behind a PV that waits on exp(sk); ACT (the exp engine, the largest
    single engine load) stays saturated.
  - PV is TRANSPOSED: lhsT = exp-scores slice [keys 128, q-tile 128]
    (stationary, full 128-wide array), rhs = va [keys, HD+1] -> psum
    ctx[q, HD+1]; the va ones column puts the softmax denominator in psum
    COLUMN 64, i.e. a per-partition scalar.
  - Normalize: DVE reciprocal on the [128, 1] denominator column +
    per-partition tensor_scalar multiply (no cross-partition scatter
    DMAs, no PE broadcast matmuls), then a PE transpose per (head,
    q-tile) puts ctx back feature-major for the out-projection; head 1
    transposes straight onto partitions 64..127 via col tile_position.
  - Cross-phase overlap via two 'pending unit' queues (PE-light normalize
    tails vs PE-heavy projection/out-projection blocks), drained one of
    each per sk-step into the ACT-paced attention loop's PE bubbles.
"""
import numpy as np

B = 2
S = 2048
D = 1024
H = 16
HD = 64
NCORES = 8
HPC = H // NCORES       # heads per core
FPC = HPC * HD          # 128 features per core


def build_mha_kernel(tc, outT, xT, wqT, wkT, wvT, woT, s=S, d=D):
    """Emit the per-core MHA program.

    outT: [B, d, s] bf16 (partial output, transposed, per-batch)
    xT:   [B, d, s] bf16
    wqT/wkT/wvT: [128, d//128, FPC] bf16, host pre-arranged so the
        weight DMA is contiguous (wqT pre-scaled by 1/sqrt(HD))
    woT:  [FPC, d] bf16
    """
    import concourse.mybir as mybir
    from contextlib import ExitStack

    nc = tc.nc
    f32 = mybir.dt.float32
    f32r = mybir.dt.float32r
    bf16 = mybir.dt.bfloat16
    Exp = mybir.ActivationFunctionType.Exp

    KT = d // 128           # contraction tiles for projections
    SK = s // 128           # key tiles per batch
    NCH = s // 512          # 512-token chunks
    NQB = s // 512          # query blocks
    QTPB = 4                # 128-wide q-tiles per 512-wide query block

    with ExitStack() as es:
        consts = es.enter_context(tc.tile_pool(name="consts", bufs=1))
        wpool = es.enter_context(tc.tile_pool(name="w", bufs=1))
        xpool = es.enter_context(tc.tile_pool(name="xt", bufs=2))
        qkv = es.enter_context(tc.tile_pool(name="qkv", bufs=2))
        vapool = es.enter_context(tc.tile_pool(name="va", bufs=1))
        epool = es.enter_context(tc.tile_pool(name="exp", bufs=2))
        cpool = es.enter_context(tc.tile_pool(name="ctxT", bufs=2))
        spool = es.enter_context(tc.tile_pool(name="small", bufs=2))
        opool = es.enter_context(tc.tile_pool(name="o", bufs=4))
        ps_sc = es.enter_context(tc.tile_pool(name="pssc", bufs=2, space="PSUM"))
        ps_ctx = es.enter_context(tc.tile_pool(name="psctx", bufs=1, space="PSUM"))
        ps_wk = es.enter_context(tc.tile_pool(name="pswk", bufs=2, space="PSUM"))

        from concourse.masks import make_identity
        identity = consts.tile([128, 128], f32, tag="ident")
        make_identity(nc, identity[:])

        # --- weights (resident whole kernel). wq+wk on the SP queue (needed
        # first); wv/wo on the Pool queue. The ACT queue is kept clear of
        # DMA dispatches so exp issue is never delayed.
        wq_sb = wpool.tile([128, KT, FPC], bf16, tag="wq")
        nc.sync.dma_start(wq_sb[:], wqT)
        wk_sb = wpool.tile([128, KT, FPC], bf16, tag="wk")
        nc.sync.dma_start(wk_sb[:], wkT)
        wv_sb = wpool.tile([128, KT, FPC], bf16, tag="wv")
        nc.gpsimd.dma_start(wv_sb[:], wvT)
        wo_sb = wpool.tile([128, d], bf16, tag="wo")
        nc.gpsimd.dma_start(wo_sb[:], woT)

        # --- va tiles: [tokens 128, head, HD+1] with a persistent ones
        # column at [:, :, HD] (written once; v columns rewritten per batch)
        vas = []
        for sk in range(SK):
            va = vapool.tile([128, HPC, HD + 1], bf16, tag=f"va{sk}")
            nc.gpsimd.memset(va[:, :, HD:HD + 1], 1.0)
            vas.append(va)

        # --- x loads. batch 0: per-(tile, chunk) pieces, chunk-major, so the
        # first k-proj chunk can start after ~1/4 of the data; batch 1: whole
        # tiles. Split across the SP and Pool HWDGE queues.
        xts = {}
        for b in range(B):
            for k in range(KT):
                xts[(b, k)] = xpool.tile([128, s], bf16, tag=f"x{k}",
                                         name=f"x{b}_{k}")
        for ch in range(NCH):
            for k in range(KT):
                eng = nc.gpsimd if (ch * KT + k) % 2 else nc.sync
                cs = slice(ch * 512, (ch + 1) * 512)
                eng.dma_start(xts[(0, k)][:, cs], xT[0, k * 128:(k + 1) * 128, cs])
        for k in range(KT):
            eng = nc.gpsimd if k % 2 else nc.sync
            eng.dma_start(xts[(1, k)][:], xT[1, k * 128:(k + 1) * 128, :])

        # ---------- unit builders (each emits a small instruction group) ----
        def proj_chunk(dst, w_sb, b, ch):
            # feat-major projection chunk: dst[:, ch*512:+512] (for q/k)
            cs = slice(ch * 512, (ch + 1) * 512)
            pt = ps_wk.tile([128, 512], f32, tag="wk")
            for k in range(KT):
                nc.tensor.matmul(pt[:], w_sb[:, k, :], xts[(b, k)][:, cs],
                                 start=(k == 0), stop=(k == KT - 1))
            nc.vector.tensor_copy(dst[:, cs], pt[:])

        def proj_chunk_halves(dst, w_sb, b, ch):
            # proj_chunk split into two pending units (halves the PE burst a
            # unit injects into the ACT-paced loop). The psum ring has 2
            # slots and at most one other unit runs between the halves, so
            # the accumulator survives; the two halves MUST stay adjacent
            # in the heavies queue.
            cs = slice(ch * 512, (ch + 1) * 512)
            state = {}

            def half1():
                pt = ps_wk.tile([128, 512], f32, tag="wk", name="pt")
                for k in range(KT // 2):
                    nc.tensor.matmul(pt[:], w_sb[:, k, :], xts[(b, k)][:, cs],
                                     start=(k == 0), stop=False)
                state["pt"] = pt

            def half2():
                pt = state.pop("pt")
                for k in range(KT // 2, KT):
                    nc.tensor.matmul(pt[:], w_sb[:, k, :], xts[(b, k)][:, cs],
                                     start=False, stop=(k == KT - 1))
                nc.vector.tensor_copy(dst[:, cs], pt[:])

            return [half1, half2]

        def vproj_unit(b, sk):
            # token-major v projection: va[sk] tokens sk*128..+128, both heads
            ts_ = slice(sk * 128, (sk + 1) * 128)
            vp = ps_wk.tile([128, 512], f32, tag="wk")
            for k in range(KT):
                nc.tensor.matmul(vp[:, 0:FPC], xts[(b, k)][:, ts_],
                                 wv_sb[:, k, :],
                                 start=(k == 0), stop=(k == KT - 1))
            src = vp[:, 0:FPC].rearrange("p (j f) -> p j f", j=HPC)
            nc.vector.tensor_copy(vas[sk][:, :, 0:HD], src)

        def emit_outproj(ctxT_b, bb, m, ch, eng):
            # one partial out-projection unit: outT[bb] tile (m, ch)
            ms = slice(m * 128, (m + 1) * 128)
            cs = slice(ch * 512, (ch + 1) * 512)
            op = ps_wk.tile([128, 512], f32, tag="wk")
            nc.tensor.matmul(op[:], wo_sb[:, ms], ctxT_b[:, cs],
                             start=True, stop=True)
            ot = opool.tile([128, 512], bf16, tag="ot")
            if eng is nc.scalar:
                eng.copy(ot[:], op[:])
            else:
                eng.tensor_copy(ot[:], op[:])
            nc.sync.dma_start(outT[bb, ms, cs], ot[:])

        # pending unit queues, drained into the attention loop's PE bubbles:
        # `lights` are PE-light normalize tails, `heavies` are PE-heavy
        # projection / out-projection blocks. One of each per sk-step.
        lights = []
        heavies = []

        def drain(n_heavy=1):
            if lights:
                lights.pop(0)()
            popped = 0
            while heavies and popped < n_heavy:
                heavies.pop(0)()
                popped += 1
            if not popped and lights:
                lights.pop(0)()

        # q/k tiles per batch: allocate both batches' ring slots up front so
        # units queued during batch 0 write the buffers batch 1 will read.
        qTs = [qkv.tile([128, s], bf16, tag="q", name=f"qT{b}")
               for b in range(B)]
        kTs = [qkv.tile([128, s], bf16, tag="k", name=f"kT{b}")
               for b in range(B)]
        ctxTs = [cpool.tile([128, s], bf16, tag="ctxT", name=f"ctxT{b}")
                 for b in range(B)]

        def norm_unit(h, cs_h, ctxT, qb, release):
            # per (head, qb): 1/denom columns for all 4 q-tiles in one DVE
            # reciprocal, per-partition multiplies on Pool (SBUF-only), PE
            # transposes back to feature-major into ONE psum tile (head 1
            # lands on partitions 64..127 via col tile_position), single
            # DVE evacuation into ctxT.
            rc = spool.tile([128, QTPB], f32, tag=f"rc{h}", name="rc")
            nc.vector.reciprocal(rc[:], cs_h[:, :, HD])
            cn = spool.tile([128, QTPB, HD], f32, tag=f"cn{h}", name="cn")
            for qt in range(QTPB):
                nc.gpsimd.tensor_scalar_mul(cn[:, qt, :], cs_h[:, qt, 0:HD],
                                            rc[:, qt:qt + 1])
            wkt = ps_wk.tile([128, 512], f32, tag="wk", name="tpw")
            tpv = wkt[0:HD, :]
            for qt in range(QTPB):
                nc.tensor.transpose(tpv[:, qt * 128:(qt + 1) * 128],
                                    cn[:, qt, :], identity[:])
            if h == 0:
                nc.vector.tensor_copy(ctxT[0:HD, qb * 512:(qb + 1) * 512],
                                      tpv)
            else:
                # transpose outputs must start at psum partition 0; head 1's
                # rows reach ctxT partitions 64..127 via an SBUF-SBUF DMA
                cn2 = spool.tile([HD, 512], bf16, tag="cn2", name="cn2")
                nc.vector.tensor_copy(cn2[:], tpv)
                nc.gpsimd.dma_start(
                    ctxT[HD:2 * HD, qb * 512:(qb + 1) * 512], cn2[:])
            if release:
                # ctxT columns for this qb are complete: release the
                # out-projection units for it
                heavies.extend(release)

        # batch 0 projections: attention can start once k-chunk0 (keys 0-511
        # = sk 0-3) and q-chunk0 are in; later k chunks are woven ahead of
        # the v-proj units into qb0's early steps (chunk c is consumed from
        # sk=4c, drained at step c-1). Steps 0..NCH-2 drain two heavies so
        # the v-proj units stay on their just-in-time schedule.
        proj_chunk(kTs[0], wk_sb, 0, 0)
        proj_chunk(qTs[0], wq_sb, 0, 0)
        weave = [(lambda ch=ch: proj_chunk(kTs[0], wk_sb, 0, ch))
                 for ch in range(1, NCH)]
        vunits = [(lambda sk=sk: vproj_unit(0, sk)) for sk in range(SK)]
        for i, u in enumerate(weave):
            vunits.insert(2 * i, u)
        heavies.extend(vunits)

        # ---------- attention ----------
        b1proj_left = [0]
        for b in range(B):
            qT, kT, ctxT = qTs[b], kTs[b], ctxTs[b]
            for qb in range(NQB):
                qs = slice(qb * 512, (qb + 1) * 512)

                # ctx accumulators: per head, 4 q-tiles x (HD+1) packed in
                # one PSUM bank ([128, 4, 128] fp32, slices [:, qt, 0:65])
                accs = [ps_ctx.tile([128, QTPB, 128], f32, tag=f"acc{h}",
                                    name=f"acc{h}")
                        for h in range(HPC)]
                ets = {}

                def emit_qk(sk, qT=qT, kT=kT, qs=qs, ets=ets):
                    sps = ps_sc.tile([128, 2 * 512], f32, tag="sc", name="sps")
                    for h in range(HPC):
                        hr = slice(h * HD, (h + 1) * HD)
                        nc.tensor.matmul(
                            sps[:, h * 512:(h + 1) * 512],
                            kT[hr, sk * 128:(sk + 1) * 128],
                            qT[hr, qs], start=True, stop=True)
                    et = epool.tile([128, 2 * 512], bf16, tag=f"et{sk % 2}",
                                    name="et")
                    nc.scalar.activation(et[:], sps[:], Exp)
                    ets[sk] = et

                def emit_pv(sk, accs=accs, ets=ets):
                    # transposed PV: exp-scores stationary (full width),
                    # va moving; ctx[q, hd] + denominator column in psum
                    et = ets.pop(sk)
                    for h in range(HPC):
                        for qt in range(QTPB):
                            # the 4 q-tile accumulators share one psum bank
                            # (= one 2KB zero region): only the bank's FIRST
                            # matmul starts (zeroing the whole region), only
                            # its LAST stops
                            nc.tensor.matmul(
                                accs[h][:, qt, 0:HD + 1],
                                et[:, h * 512 + qt * 128:
                                   h * 512 + (qt + 1) * 128],
                                vas[sk][:, h, :],
                                start=(sk == 0 and qt == 0),
                                stop=(sk == SK - 1 and qt == QTPB - 1))

                # software-pipelined sk loop: QK(sk+1) lands before PV(sk),
                # pending units fill the gap where PV waits on exp.
                emit_qk(0)
                if b == 0 and qb + 1 < NQB:
                    # next q chunk; sits behind QK(0) so exp(0) starts on time
                    proj_chunk(qT, wq_sb, 0, qb + 1)
                for sk in range(SK):
                    if sk + 1 < SK:
                        emit_qk(sk + 1)
                    drain(2 if (b == 0 and qb == 0 and sk < NCH - 1) else 1)
                    emit_pv(sk)

                # evacuate the raw accumulators to SBUF right away (frees
                # the psum banks for the next qb); normalize tails become
                # PE-light pending units.
                css = []
                for h in range(HPC):
                    cs_h = spool.tile([128, QTPB, HD + 1], f32, tag=f"cs{h}",
                                      name=f"cs{h}")
                    nc.vector.tensor_copy(cs_h[:], accs[h][:, :, 0:HD + 1])
                    css.append(cs_h)

                # out-projection units for this qb, released by the last
                # normalize unit (they read the ctxT columns it completes).
                # The final qb's units run in the drain tail where ACT is
                # idle: alternate their psum evacuations DVE/ACT there.
                tail_qb = (b == B - 1 and qb == NQB - 1)
                opr = [(lambda c=ctxT, bb=b, mm=m, cc=qb,
                        e=(nc.scalar if (tail_qb and m % 2) else nc.vector):
                        emit_outproj(c, bb, mm, cc, e))
                       for m in range(KT)]
                # head 1 first: its ctxT rows travel by SBUF-SBUF DMA, so
                # putting it ahead lets that latency overlap head 0's work
                for h in reversed(range(HPC)):
                    lights.append(
                        lambda h=h, cs_h=css[h], ctxT=ctxT, qb=qb,
                        release=(opr if h == 0 else None):
                            norm_unit(h, cs_h, ctxT, qb, release))

                if b == 0 and qb == min(1, NQB - 2):
                    # batch 1's k projection + all q chunks, into the tail
                    # of batch 0's attention (the qkv ring slots for batch 1
                    # are free; x(1) tiles have loaded long since)
                    def done(u):
                        def f():
                            u()
                            b1proj_left[0] -= 1
                        return f
                    units = []
                    for ch in range(NCH):
                        units += proj_chunk_halves(kTs[1], wk_sb, 1, ch)
                    for ch in range(NCH):
                        units += proj_chunk_halves(qTs[1], wq_sb, 1, ch)
                    b1proj_left[0] = len(units)
                    heavies.extend(done(u) for u in units)

            if b == 0:
                # make sure batch 1's q/k are in place before its attention
                # emits reads of them (no-op at full size: they drained
                # into qb2/qb3's bubbles already)
                while b1proj_left[0] > 0:
                    drain()
                # batch 1's v tiles refill inside batch 1's qb0 bubbles,
                # ahead of everything else queued (its PV needs va[sk] by
                # step sk)
                heavies[0:0] = [
                    (lambda sk=sk: vproj_unit(1, sk)) for sk in range(SK)]

        # drain everything left (batch-1 norm tails + outproj backlog)
        while lights or heavies:
            drain()


_CACHE = {}


def _get_compiled(s=S, d=D, reps=1):
    key = (s, d, reps)
    if key not in _CACHE:
        import concourse.bacc as bacc
        import concourse.tile as tile
        import concourse.mybir as mybir

        bf16 = mybir.dt.bfloat16
        nc = bacc.Bacc("TRN2", target_bir_lowering=False, debug=False)
        xT = nc.dram_tensor("xT", [B, d, s], bf16, kind="ExternalInput")
        wqT = nc.dram_tensor("wqT", [128, d // 128, FPC], bf16,
                             kind="ExternalInput")
        wkT = nc.dram_tensor("wkT", [128, d // 128, FPC], bf16,
                             kind="ExternalInput")
        wvT = nc.dram_tensor("wvT", [128, d // 128, FPC], bf16,
                             kind="ExternalInput")
        woT = nc.dram_tensor("woT", [FPC, d], bf16, kind="ExternalInput")
        outT = nc.dram_tensor("outT", [B, d, s], bf16, kind="ExternalOutput")
        with tile.TileContext(nc) as tc:
            for _ in range(reps):
                build_mha_kernel(tc, outT.ap(), xT.ap(), wqT.ap(), wkT.ap(),
                                 wvT.ap(), woT.ap(), s=s, d=d)
        nc.compile()
        _CACHE[key] = nc
    return _CACHE[key]


def _bf16(a):
    import ml_dtypes
    return np.ascontiguousarray(np.asarray(a, dtype=np.float32)).astype(
        ml_dtypes.bfloat16)


def make_in_maps(x, Wq, Wk, Wv, Wo):
    """Host-side shard prep: transpose x, slice + transpose weights per core."""
    b, s, d = x.shape
    xT = _bf16(x.transpose(0, 2, 1))
    scale = np.float32(1.0 / np.sqrt(HD))

    def prearr(wt):
        # [d, FPC] -> [128, d//128, FPC] so each SBUF partition row is one
        # contiguous DMA line (avoids 512B-descriptor strided reads)
        return _bf16(wt.reshape(d // 128, 128, FPC).transpose(1, 0, 2))

    in_maps = []
    for c in range(NCORES):
        if (c + 1) * FPC > d:
            # small-D sim configs: fewer head-slices than cores
            in_maps.append(in_maps[0])
            continue
        rs = slice(c * FPC, (c + 1) * FPC)
        in_maps.append({
            "xT": xT,
            "wqT": prearr((Wq[rs, :] * scale).T.astype(np.float32)),
            "wkT": prearr(Wk[rs, :].T.astype(np.float32)),
            "wvT": prearr(Wv[rs, :].T.astype(np.float32)),
            "woT": _bf16(Wo[:, rs].T),
        })
    return in_maps


_RUNNER = None
_RUNNER_STATE = {}


def _get_runner():
    """Build (once) a cached jitted SPMD executor mirroring
    bass2jax.run_bass_via_pjrt's multi-core path."""
    global _RUNNER
    if _RUNNER is None:
        import jax
        import jax.numpy as jnp
        from jax.sharding import Mesh, PartitionSpec
        from jax.experimental.shard_map import shard_map
        import concourse.mybir as mybir
        from concourse import bass2jax

        nc = _get_compiled()
        bass2jax.install_neuronx_cc_hook()

        partition_name = (nc.partition_id_tensor.name
                          if nc.partition_id_tensor else None)
        in_names = []
        out_names = []
        out_avals = []
        for alloc in nc.m.functions[0].allocations:
            if not isinstance(alloc, mybir.MemoryLocationSet):
                continue
            name = alloc.memorylocations[0].name
            if alloc.kind == "ExternalInput":
                if name != partition_name:
                    in_names.append(name)
            elif alloc.kind == "ExternalOutput":
                out_names.append(name)
                out_avals.append(jax.core.ShapedArray(
                    tuple(alloc.tensor_shape), mybir.dt.np(alloc.dtype)))
        n_outs = len(out_names)
        all_names = in_names + out_names
        if partition_name is not None:
            all_names = all_names + [partition_name]

        def _body(*args):
            operands = list(args)
            if partition_name is not None:
                operands.append(bass2jax.partition_id_tensor())
            outs = bass2jax._bass_exec_p.bind(
                *operands,
                out_avals=tuple(out_avals),
                in_names=tuple(all_names),
                out_names=tuple(out_names),
                lowering_input_output_aliases=(),
                sim_require_finite=True,
                sim_require_nnan=True,
                nc=nc,
            )
            return tuple(outs)

        devices = jax.devices()[:NCORES]
        mesh = Mesh(np.asarray(devices), ("core",))
        # xT is identical on every core: replicate it instead of concatenating
        # 8 copies on the host.
        in_specs = tuple(PartitionSpec() if name == "xT" else PartitionSpec("core")
                         for name in in_names)
        sharded = jax.jit(
            shard_map(_body, mesh=mesh,
                      in_specs=in_specs + (PartitionSpec("core"),) * n_outs,
                      out_specs=(PartitionSpec("core"),) * n_outs,
                      check_rep=False),
            keep_unused=True)

        # separate jit: on-device sum of the 8 per-core partials (all-reduce)
        def _reduce(a):
            return jnp.sum(a.reshape((NCORES,) + tuple(out_avals[0].shape))
                           .astype(jnp.float32), axis=0)
        reduce_jit = jax.jit(_reduce)

        out_shapes = [tuple(a.shape) for a in out_avals]
        out_dtypes = [a.dtype for a in out_avals]
        zeros_dev = [None]

        from jax.sharding import NamedSharding
        rep_shd = NamedSharding(mesh, PartitionSpec())

        def call(in_maps):
            args = []
            for name in in_names:
                if name == "xT":
                    # one host->device transfer, then device-side broadcast
                    xd = jax.device_put(np.asarray(in_maps[0][name]),
                                        devices[0])
                    args.append(jax.device_put(xd, rep_shd))
                else:
                    args.append(np.concatenate(
                        [np.asarray(m[name]) for m in in_maps], axis=0))
            if zeros_dev[0] is None:
                from jax.sharding import NamedSharding
                shd = NamedSharding(mesh, PartitionSpec("core"))
                zeros_dev[0] = [
                    jax.device_put(
                        np.zeros((NCORES * sh[0],) + sh[1:], dt), shd)
                    for sh, dt in zip(out_shapes, out_dtypes)]
            outs = sharded(*args, *zeros_dev[0])
            try:
                summed = np.asarray(reduce_jit(outs[0]))
            except Exception:
                # device reduce unavailable: fetch partials, sum on host
                a = np.asarray(outs[0]).astype(np.float64)
                summed = a.reshape((NCORES,) + tuple(out_avals[0].shape)).sum(0)
            return {out_names[0]: summed}

        _RUNNER_STATE.update(sharded=sharded, in_names=in_names,
                             out_shapes=out_shapes, out_dtypes=out_dtypes,
                             call=call, mesh=mesh)
        _RUNNER = call
    return _RUNNER


def run(x, Wq, Wk, Wv, Wo, bo, trace=False):
    from concourse._compat import axon_active
    in_maps = make_in_maps(x, Wq, Wk, Wv, Wo)
    if axon_active():
        summed = _get_runner()(in_maps)
        acc = summed["outT"].astype(np.float64)
        results = summed
    else:
        # native /dev/neuron* path (non-axon environments)
        from concourse import bass_utils
        r = bass_utils.run_bass_kernel_spmd(
            _get_compiled(), in_maps, core_ids=list(range(NCORES)), trace=trace)
        results = r.results
        acc = np.zeros((B, D, S), dtype=np.float64)
        for c in range(NCORES):
            acc += np.asarray(results[c]["outT"], dtype=np.float64)
    out = acc.transpose(0, 2, 1) + np.asarray(bo, dtype=np.float64)
    return out.astype(np.float32), results


def kernel(x, Wq, Wk, Wv, Wo, bo):
    out, _ = run(np.asarray(x), np.asarray(Wq), np.asarray(Wk),
                 np.asarray(Wv), np.asarray(Wo), np.asarray(bo))
    return out


# revision 46
# speedup vs baseline: 1.3469x; 1.0250x over previous
"""Multi-head attention (B=2, S=2048, D=1024, H=16, no mask) on 8 TRN2 cores.

Sharding: tensor-parallel over heads — 2 heads per core. Each core computes
its heads' QKV projections, attention, and a partial out-projection
(row-sharded Wo); the host sums the 8 partials and adds the bias (the
all-reduce happens at gather time).

Device layout (per core), v3 — bf16 dataflow, software-pipelined attention,
transposed PV with per-partition softmax normalization:
  - All tensor data bf16 (x, W, q/k/v, exp-scores, ctx, partial out);
    matmul accumulation and softmax statistics stay fp32 in PSUM.
  - qT/kT kept transposed (feat-on-partitions); v projected DIRECTLY in
    token-major layout ([tokens, head, HD+1] va tiles, ones column
    prebaked) via per-token-tile matmuls — no PE transpose pass for v.
  - scoresT[k, q] = k @ qT per (head, key-tile): the two heads' K=64
    matmuls sit on array row-groups 0/64 and run concurrently on HW.
  - The attention inner loop is software-pipelined: QK(sk+1) is emitted
    BEFORE PV(sk), so the in-order PE never parks the next score matmul
    behind a PV that waits on exp(sk); ACT (the exp engine, the largest
    single engine load) stays saturated.
  - PV is TRANSPOSED: lhsT = exp-scores slice [keys 128, q-tile 128]
    (stationary, full 128-wide array), rhs = va [keys, HD+1] -> psum
    ctx[q, HD+1]; the va ones column puts the softmax denominator in psum
    COLUMN 64, i.e. a per-partition scalar.
  - Normalize: DVE reciprocal on the [128, 1] denominator column +
    per-partition tensor_scalar multiply (no cross-partition scatter
    DMAs, no PE broadcast matmuls), then a PE transpose per (head,
    q-tile) puts ctx back feature-major for the out-projection; head 1
    transposes straight onto partitions 64..127 via col tile_position.
  - Cross-phase overlap via two 'pending unit' queues (PE-light normalize
    tails vs PE-heavy projection/out-projection blocks), drained one of
    each per sk-step into the ACT-paced attention loop's PE bubbles.
"""
import numpy as np

B = 2
S = 2048
D = 1024
H = 16
HD = 64
NCORES = 8
HPC = H // NCORES       # heads per core
FPC = HPC * HD          # 128 features per core


def build_mha_kernel(tc, outT, xT, wqT, wkT, wvT, woT, s=S, d=D):
    """Emit the per-core MHA program.

    outT: [B, d, s] bf16 (partial output, transposed, per-batch)
    xT:   [B, d, s] bf16
    wqT/wkT/wvT: [128, d//128, FPC] bf16, host pre-arranged so the
        weight DMA is contiguous (wqT pre-scaled by 1/sqrt(HD))
    woT:  [FPC, d] bf16
    """
    import concourse.mybir as mybir
    from contextlib import ExitStack

    nc = tc.nc
    f32 = mybir.dt.float32
    f32r = mybir.dt.float32r
    bf16 = mybir.dt.bfloat16
    Exp = mybir.ActivationFunctionType.Exp

    KT = d // 128           # contraction tiles for projections
    SK = s // 128           # key tiles per batch
    NCH = s // 512          # 512-token chunks
    NQB = s // 512          # query blocks
    QTPB = 4                # 128-wide q-tiles per 512-wide query block

    with ExitStack() as es:
        consts = es.enter_context(tc.tile_pool(name="consts", bufs=1))
        wpool = es.enter_context(tc.tile_pool(name="w", bufs=1))
        xpool = es.enter_context(tc.tile_pool(name="xt", bufs=2))
        qkv = es.enter_context(tc.tile_pool(name="qkv", bufs=2))
        vapool = es.enter_context(tc.tile_pool(name="va", bufs=1))
        epool = es.enter_context(tc.tile_pool(name="exp", bufs=2))
        cpool = es.enter_context(tc.tile_pool(name="ctxT", bufs=2))
        spool = es.enter_context(tc.tile_pool(name="small", bufs=2))
        opool = es.enter_context(tc.tile_pool(name="o", bufs=4))
        ps_sc = es.enter_context(tc.tile_pool(name="pssc", bufs=2, space="PSUM"))
        ps_ctx = es.enter_context(tc.tile_pool(name="psctx", bufs=1, space="PSUM"))
        ps_wk = es.enter_context(tc.tile_pool(name="pswk", bufs=2, space="PSUM"))

        from concourse.masks import make_identity
        identity = consts.tile([128, 128], f32, tag="ident")
        make_identity(nc, identity[:])

        # --- weights (resident whole kernel). wq+wk on the SP queue (needed
        # first); wv/wo on the Pool queue. The ACT queue is kept clear of
        # DMA dispatches so exp issue is never delayed.
        wq_sb = wpool.tile([128, KT, FPC], bf16, tag="wq")
        nc.sync.dma_start(wq_sb[:], wqT)
        wk_sb = wpool.tile([128, KT, FPC], bf16, tag="wk")
        nc.sync.dma_start(wk_sb[:], wkT)
        wv_sb = wpool.tile([128, KT, FPC], bf16, tag="wv")
        nc.gpsimd.dma_start(wv_sb[:], wvT)
        wo_sb = wpool.tile([128, d], bf16, tag="wo")
        nc.gpsimd.dma_start(wo_sb[:], woT)

        # --- va tiles: [tokens 128, head, HD+1] with a persistent ones
        # column at [:, :, HD] (written once; v columns rewritten per batch)
        vas = []
        for sk in range(SK):
            va = vapool.tile([128, HPC, HD + 1], bf16, tag=f"va{sk}")
            nc.gpsimd.memset(va[:, :, HD:HD + 1], 1.0)
            vas.append(va)

        # --- x loads. batch 0: per-(tile, chunk) pieces, chunk-major, so the
        # first k-proj chunk can start after ~1/4 of the data; batch 1: whole
        # tiles. Split across the SP and Pool HWDGE queues.
        xts = {}
        for b in range(B):
            for k in range(KT):
                xts[(b, k)] = xpool.tile([128, s], bf16, tag=f"x{k}",
                                         name=f"x{b}_{k}")
        for ch in range(NCH):
            for k in range(KT):
                eng = nc.gpsimd if (ch * KT + k) % 2 else nc.sync
                cs = slice(ch * 512, (ch + 1) * 512)
                eng.dma_start(xts[(0, k)][:, cs], xT[0, k * 128:(k + 1) * 128, cs])
        for k in range(KT):
            eng = nc.gpsimd if k % 2 else nc.sync
            eng.dma_start(xts[(1, k)][:], xT[1, k * 128:(k + 1) * 128, :])

        # ---------- unit builders (each emits a small instruction group) ----
        def proj_chunk(dst, w_sb, b, ch):
            # feat-major projection chunk: dst[:, ch*512:+512] (for q/k)
            cs = slice(ch * 512, (ch + 1) * 512)
            pt = ps_wk.tile([128, 512], f32, tag="wk")
            for k in range(KT):
                nc.tensor.matmul(pt[:], w_sb[:, k, :], xts[(b, k)][:, cs],
                                 start=(k == 0), stop=(k == KT - 1))
            nc.vector.tensor_copy(dst[:, cs], pt[:])

        def proj_chunk_halves(dst, w_sb, b, ch):
            # proj_chunk split into two pending units (halves the PE burst a
            # unit injects into the ACT-paced loop). The psum ring has 2
            # slots and at most one other unit runs between the halves, so
            # the accumulator survives; the two halves MUST stay adjacent
            # in the heavies queue.
            cs = slice(ch * 512, (ch + 1) * 512)
            state = {}

            def half1():
                pt = ps_wk.tile([128, 512], f32, tag="wk", name="pt")
                for k in range(KT // 2):
                    nc.tensor.matmul(pt[:], w_sb[:, k, :], xts[(b, k)][:, cs],
                                     start=(k == 0), stop=False)
                state["pt"] = pt

            def half2():
                pt = state.pop("pt")
                for k in range(KT // 2, KT):
                    nc.tensor.matmul(pt[:], w_sb[:, k, :], xts[(b, k)][:, cs],
                                     start=False, stop=(k == KT - 1))
                nc.vector.tensor_copy(dst[:, cs], pt[:])

            return [half1, half2]

        def vproj_unit(b, sk):
            # token-major v projection: va[sk] tokens sk*128..+128, both heads
            ts_ = slice(sk * 128, (sk + 1) * 128)
            vp = ps_wk.tile([128, 512], f32, tag="wk")
            for k in range(KT):
                nc.tensor.matmul(vp[:, 0:FPC], xts[(b, k)][:, ts_],
                                 wv_sb[:, k, :],
                                 start=(k == 0), stop=(k == KT - 1))
            src = vp[:, 0:FPC].rearrange("p (j f) -> p j f", j=HPC)
            nc.vector.tensor_copy(vas[sk][:, :, 0:HD], src)

        def emit_outproj(ctxT_b, bb, m, ch, eng, split=False):
            # one partial out-projection unit: outT[bb] tile (m, ch)
            ms = slice(m * 128, (m + 1) * 128)
            cs = slice(ch * 512, (ch + 1) * 512)
            op = ps_wk.tile([128, 512], f32, tag="wk")
            if split:
                # tail variant: head 1's context read from its SBUF staging
                # tile (partitions 0..63) via a second K=64 matmul
                nc.tensor.matmul(op[:], wo_sb[0:HD, ms], ctxT_b[0:HD, cs],
                                 start=True, stop=False)
                nc.tensor.matmul(op[:], wo_hi[:, ms], cn2_stash[ch][:],
                                 start=False, stop=True)
            else:
                nc.tensor.matmul(op[:], wo_sb[:, ms], ctxT_b[:, cs],
                                 start=True, stop=True)
            ot = opool.tile([128, 512], bf16, tag="ot")
            if eng is nc.scalar:
                eng.copy(ot[:], op[:])
            else:
                eng.tensor_copy(ot[:], op[:])
            nc.sync.dma_start(outT[bb, ms, cs], ot[:])

        # pending unit queues, drained into the attention loop's PE bubbles:
        # `lights` are PE-light normalize tails, `heavies` are PE-heavy
        # projection / out-projection blocks. One of each per sk-step.
        lights = []
        heavies = []

        def drain(n_heavy=1):
            if lights:
                lights.pop(0)()
            popped = 0
            while heavies and popped < n_heavy:
                heavies.pop(0)()
                popped += 1
            if not popped and lights:
                lights.pop(0)()

        # q/k tiles per batch: allocate both batches' ring slots up front so
        # units queued during batch 0 write the buffers batch 1 will read.
        qTs = [qkv.tile([128, s], bf16, tag="q", name=f"qT{b}")
               for b in range(B)]
        kTs = [qkv.tile([128, s], bf16, tag="k", name=f"kT{b}")
               for b in range(B)]
        ctxTs = [cpool.tile([128, s], bf16, tag="ctxT", name=f"ctxT{b}")
                 for b in range(B)]

        # second copy of wo's high rows at base partition 0: lets the drain
        # tail's out-projections take head 1's context from SBUF directly
        # (two K=64 accumulating matmuls) instead of waiting on the
        # cross-partition DMA into ctxT
        wo_hi = wpool.tile([HD, d], bf16, tag="wo_hi")
        nc.vector.tensor_copy(wo_hi[:], wo_sb[HD:2 * HD, :])
        cn2_stash = {}

        def norm_unit(h, cs_h, ctxT, qb, release, skip_dma=False):
            # per (head, qb): 1/denom columns for all 4 q-tiles in one DVE
            # reciprocal, per-partition multiplies on Pool (SBUF-only), PE
            # transposes back to feature-major into ONE psum tile (head 1
            # lands on partitions 64..127 via col tile_position), single
            # DVE evacuation into ctxT.
            rc = spool.tile([128, QTPB], f32, tag=f"rc{h}", name="rc")
            nc.vector.reciprocal(rc[:], cs_h[:, :, HD])
            cn = spool.tile([128, QTPB, HD], f32, tag=f"cn{h}", name="cn")
            for qt in range(QTPB):
                nc.gpsimd.tensor_scalar_mul(cn[:, qt, :], cs_h[:, qt, 0:HD],
                                            rc[:, qt:qt + 1])
            wkt = ps_wk.tile([128, 512], f32, tag="wk", name="tpw")
            tpv = wkt[0:HD, :]
            for qt in range(QTPB):
                nc.tensor.transpose(tpv[:, qt * 128:(qt + 1) * 128],
                                    cn[:, qt, :], identity[:])
            if h == 0:
                nc.vector.tensor_copy(ctxT[0:HD, qb * 512:(qb + 1) * 512],
                                      tpv)
            else:
                # transpose outputs must start at psum partition 0; head 1's
                # rows reach ctxT partitions 64..127 via an SBUF-SBUF DMA
                cn2 = spool.tile([HD, 512], bf16, tag="cn2", name="cn2")
                nc.vector.tensor_copy(cn2[:], tpv)
                if skip_dma:
                    cn2_stash[qb] = cn2
                else:
                    nc.gpsimd.dma_start(
                        ctxT[HD:2 * HD, qb * 512:(qb + 1) * 512], cn2[:])
            if release:
                # ctxT columns for this qb are complete: release the
                # out-projection units for it
                heavies.extend(release)

        # batch 0 projections: attention can start once k-chunk0 (keys 0-511
        # = sk 0-3) and q-chunk0 are in; later k chunks are woven ahead of
        # the v-proj units into qb0's early steps (chunk c is consumed from
        # sk=4c, drained at step c-1). Steps 0..NCH-2 drain two heavies so
        # the v-proj units stay on their just-in-time schedule.
        proj_chunk(kTs[0], wk_sb, 0, 0)
        proj_chunk(qTs[0], wq_sb, 0, 0)
        weave = [(lambda ch=ch: proj_chunk(kTs[0], wk_sb, 0, ch))
                 for ch in range(1, NCH)]
        vunits = [(lambda sk=sk: vproj_unit(0, sk)) for sk in range(SK)]
        for i, u in enumerate(weave):
            vunits.insert(2 * i, u)
        heavies.extend(vunits)

        # ---------- attention ----------
        b1proj_left = [0]
        qk_pre = {}
        for b in range(B):
            qT, kT, ctxT = qTs[b], kTs[b], ctxTs[b]
            for qb in range(NQB):
                qs = slice(qb * 512, (qb + 1) * 512)

                # ctx accumulators: per head, 4 q-tiles x (HD+1) packed in
                # one PSUM bank ([128, 4, 128] fp32, slices [:, qt, 0:65])
                accs = [ps_ctx.tile([128, QTPB, 128], f32, tag=f"acc{h}",
                                    name=f"acc{h}")
                        for h in range(HPC)]
                ets = {}

                def emit_qk(sk, qT=qT, kT=kT, qs=qs, ets=ets, store=None):
                    sps = ps_sc.tile([128, 2 * 512], f32, tag="sc", name="sps")
                    for h in range(HPC):
                        hr = slice(h * HD, (h + 1) * HD)
                        nc.tensor.matmul(
                            sps[:, h * 512:(h + 1) * 512],
                            kT[hr, sk * 128:(sk + 1) * 128],
                            qT[hr, qs], start=True, stop=True)
                    et = epool.tile([128, 2 * 512], bf16, tag=f"et{sk % 2}",
                                    name="et")
                    nc.scalar.activation(et[:], sps[:], Exp)
                    if store is not None:
                        qk_pre[store] = et
                    else:
                        ets[sk] = et

                def emit_pv(sk, accs=accs, ets=ets):
                    # transposed PV: exp-scores stationary (full width),
                    # va moving; ctx[q, hd] + denominator column in psum
                    et = ets.pop(sk)
                    for h in range(HPC):
                        for qt in range(QTPB):
                            # the 4 q-tile accumulators share one psum bank
                            # (= one 2KB zero region): only the bank's FIRST
                            # matmul starts (zeroing the whole region), only
                            # its LAST stops
                            nc.tensor.matmul(
                                accs[h][:, qt, 0:HD + 1],
                                et[:, h * 512 + qt * 128:
                                   h * 512 + (qt + 1) * 128],
                                vas[sk][:, h, :],
                                start=(sk == 0 and qt == 0),
                                stop=(sk == SK - 1 and qt == QTPB - 1))

                # software-pipelined sk loop: QK(sk+1) lands before PV(sk),
                # pending units fill the gap where PV waits on exp. The
                # FIRST QK of this qb may have been pre-emitted in the
                # previous qb's last step (qk_pre holds its exp tile).
                if qb in qk_pre:
                    ets[0] = qk_pre.pop(qb)
                else:
                    emit_qk(0)
                if b == 0 and qb + 1 < NQB:
                    # next q chunk, as two PE-light front-of-queue units
                    # (not needed until the next qb; emitting directly here
                    # would stall exp behind 1.7us of projection)
                    qh = proj_chunk_halves(qT, wq_sb, 0, qb + 1)
                    lights.insert(0, qh[0])
                    lights.insert(1, qh[1])
                for sk in range(SK):
                    if sk + 1 < SK:
                        emit_qk(sk + 1)
                    elif qb + 1 < NQB:
                        # pre-emit the next qb's first QK so its exp starts
                        # right after this qb's last one
                        emit_qk(0, qs=slice((qb + 1) * 512, (qb + 2) * 512),
                                store=(qb + 1))
                    drain(2 if (b == 0 and qb == 0 and sk < NCH - 1) else 1)
                    emit_pv(sk)

                # evacuate the raw accumulators to SBUF right away (frees
                # the psum banks for the next qb); normalize tails become
                # PE-light pending units.
                css = []
                for h in range(HPC):
                    cs_h = spool.tile([128, QTPB, HD + 1], f32, tag=f"cs{h}",
                                      name=f"cs{h}")
                    nc.vector.tensor_copy(cs_h[:], accs[h][:, :, 0:HD + 1])
                    css.append(cs_h)

                # out-projection units for this qb, released by the last
                # normalize unit (they read the ctxT columns it completes).
                # The final qb's units run in the drain tail where ACT is
                # idle: alternate their psum evacuations DVE/ACT there.
                tail_qb = (b == B - 1 and qb == NQB - 1)
                opr = [(lambda c=ctxT, bb=b, mm=m, cc=qb, sp=tail_qb,
                        e=(nc.scalar if (tail_qb and m % 2) else nc.vector):
                        emit_outproj(c, bb, mm, cc, e, split=sp))
                       for m in range(KT)]
                # head 1 first: its ctxT rows travel by SBUF-SBUF DMA, so
                # putting it ahead lets that latency overlap head 0's work
                for h in reversed(range(HPC)):
                    lights.append(
                        lambda h=h, cs_h=css[h], ctxT=ctxT, qb=qb,
                        release=(opr if h == 0 else None), sd=tail_qb:
                            norm_unit(h, cs_h, ctxT, qb, release,
                                      skip_dma=(sd and h == 1)))

                if b == 0 and qb == min(1, NQB - 2):
                    # batch 1's k projection + all q chunks, into the tail
                    # of batch 0's attention (the qkv ring slots for batch 1
                    # are free; x(1) tiles have loaded long since)
                    def done(u):
                        def f():
                            u()
                            b1proj_left[0] -= 1
                        return f
                    units = []
                    for ch in range(NCH):
                        units += proj_chunk_halves(kTs[1], wk_sb, 1, ch)
                    for ch in range(NCH):
                        units += proj_chunk_halves(qTs[1], wq_sb, 1, ch)
                    b1proj_left[0] = len(units)
                    heavies.extend(done(u) for u in units)

            if b == 0:
                # make sure batch 1's q/k are in place before its attention
                # emits reads of them (no-op at full size: they drained
                # into qb2/qb3's bubbles already)
                while b1proj_left[0] > 0:
                    drain()
                # batch 1's v tiles refill inside batch 1's qb0 bubbles,
                # ahead of everything else queued (its PV needs va[sk] by
                # step sk)
                heavies[0:0] = [
                    (lambda sk=sk: vproj_unit(1, sk)) for sk in range(SK)]

        # drain everything left (batch-1 norm tails + outproj backlog)
        while lights or heavies:
            drain()


_CACHE = {}


def _get_compiled(s=S, d=D, reps=1):
    key = (s, d, reps)
    if key not in _CACHE:
        import concourse.bacc as bacc
        import concourse.tile as tile
        import concourse.mybir as mybir

        bf16 = mybir.dt.bfloat16
        nc = bacc.Bacc("TRN2", target_bir_lowering=False, debug=False)
        xT = nc.dram_tensor("xT", [B, d, s], bf16, kind="ExternalInput")
        wqT = nc.dram_tensor("wqT", [128, d // 128, FPC], bf16,
                             kind="ExternalInput")
        wkT = nc.dram_tensor("wkT", [128, d // 128, FPC], bf16,
                             kind="ExternalInput")
        wvT = nc.dram_tensor("wvT", [128, d // 128, FPC], bf16,
                             kind="ExternalInput")
        woT = nc.dram_tensor("woT", [FPC, d], bf16, kind="ExternalInput")
        outT = nc.dram_tensor("outT", [B, d, s], bf16, kind="ExternalOutput")
        with tile.TileContext(nc) as tc:
            for _ in range(reps):
                build_mha_kernel(tc, outT.ap(), xT.ap(), wqT.ap(), wkT.ap(),
                                 wvT.ap(), woT.ap(), s=s, d=d)
        nc.compile()
        _CACHE[key] = nc
    return _CACHE[key]


def _bf16(a):
    import ml_dtypes
    return np.ascontiguousarray(np.asarray(a, dtype=np.float32)).astype(
        ml_dtypes.bfloat16)


def make_in_maps(x, Wq, Wk, Wv, Wo):
    """Host-side shard prep: transpose x, slice + transpose weights per core."""
    b, s, d = x.shape
    xT = _bf16(x.transpose(0, 2, 1))
    scale = np.float32(1.0 / np.sqrt(HD))

    def prearr(wt):
        # [d, FPC] -> [128, d//128, FPC] so each SBUF partition row is one
        # contiguous DMA line (avoids 512B-descriptor strided reads)
        return _bf16(wt.reshape(d // 128, 128, FPC).transpose(1, 0, 2))

    in_maps = []
    for c in range(NCORES):
        if (c + 1) * FPC > d:
            # small-D sim configs: fewer head-slices than cores
            in_maps.append(in_maps[0])
            continue
        rs = slice(c * FPC, (c + 1) * FPC)
        in_maps.append({
            "xT": xT,
            "wqT": prearr((Wq[rs, :] * scale).T.astype(np.float32)),
            "wkT": prearr(Wk[rs, :].T.astype(np.float32)),
            "wvT": prearr(Wv[rs, :].T.astype(np.float32)),
            "woT": _bf16(Wo[:, rs].T),
        })
    return in_maps


_RUNNER = None
_RUNNER_STATE = {}


def _get_runner():
    """Build (once) a cached jitted SPMD executor mirroring
    bass2jax.run_bass_via_pjrt's multi-core path."""
    global _RUNNER
    if _RUNNER is None:
        import jax
        import jax.numpy as jnp
        from jax.sharding import Mesh, PartitionSpec
        from jax.experimental.shard_map import shard_map
        import concourse.mybir as mybir
        from concourse import bass2jax

        nc = _get_compiled()
        bass2jax.install_neuronx_cc_hook()

        partition_name = (nc.partition_id_tensor.name
                          if nc.partition_id_tensor else None)
        in_names = []
        out_names = []
        out_avals = []
        for alloc in nc.m.functions[0].allocations:
            if not isinstance(alloc, mybir.MemoryLocationSet):
                continue
            name = alloc.memorylocations[0].name
            if alloc.kind == "ExternalInput":
                if name != partition_name:
                    in_names.append(name)
            elif alloc.kind == "ExternalOutput":
                out_names.append(name)
                out_avals.append(jax.core.ShapedArray(
                    tuple(alloc.tensor_shape), mybir.dt.np(alloc.dtype)))
        n_outs = len(out_names)
        all_names = in_names + out_names
        if partition_name is not None:
            all_names = all_names + [partition_name]

        def _body(*args):
            operands = list(args)
            if partition_name is not None:
                operands.append(bass2jax.partition_id_tensor())
            outs = bass2jax._bass_exec_p.bind(
                *operands,
                out_avals=tuple(out_avals),
                in_names=tuple(all_names),
                out_names=tuple(out_names),
                lowering_input_output_aliases=(),
                sim_require_finite=True,
                sim_require_nnan=True,
                nc=nc,
            )
            return tuple(outs)

        devices = jax.devices()[:NCORES]
        mesh = Mesh(np.asarray(devices), ("core",))
        # xT is identical on every core: replicate it instead of concatenating
        # 8 copies on the host.
        in_specs = tuple(PartitionSpec() if name == "xT" else PartitionSpec("core")
                         for name in in_names)
        sharded = jax.jit(
            shard_map(_body, mesh=mesh,
                      in_specs=in_specs + (PartitionSpec("core"),) * n_outs,
                      out_specs=(PartitionSpec("core"),) * n_outs,
                      check_rep=False),
            keep_unused=True)

        # separate jit: on-device sum of the 8 per-core partials (all-reduce)
        def _reduce(a):
            return jnp.sum(a.reshape((NCORES,) + tuple(out_avals[0].shape))
                           .astype(jnp.float32), axis=0)
        reduce_jit = jax.jit(_reduce)

        out_shapes = [tuple(a.shape) for a in out_avals]
        out_dtypes = [a.dtype for a in out_avals]
        zeros_dev = [None]

        from jax.sharding import NamedSharding
        rep_shd = NamedSharding(mesh, PartitionSpec())

        def call(in_maps):
            args = []
            for name in in_names:
                if name == "xT":
                    # one host->device transfer, then device-side broadcast
                    xd = jax.device_put(np.asarray(in_maps[0][name]),
                                        devices[0])
                    args.append(jax.device_put(xd, rep_shd))
                else:
                    args.append(np.concatenate(
                        [np.asarray(m[name]) for m in in_maps], axis=0))
            if zeros_dev[0] is None:
                from jax.sharding import NamedSharding
                shd = NamedSharding(mesh, PartitionSpec("core"))
                zeros_dev[0] = [
                    jax.device_put(
                        np.zeros((NCORES * sh[0],) + sh[1:], dt), shd)
                    for sh, dt in zip(out_shapes, out_dtypes)]
            outs = sharded(*args, *zeros_dev[0])
            try:
                summed = np.asarray(reduce_jit(outs[0]))
            except Exception:
                # device reduce unavailable: fetch partials, sum on host
                a = np.asarray(outs[0]).astype(np.float64)
                summed = a.reshape((NCORES,) + tuple(out_avals[0].shape)).sum(0)
            return {out_names[0]: summed}

        _RUNNER_STATE.update(sharded=sharded, in_names=in_names,
                             out_shapes=out_shapes, out_dtypes=out_dtypes,
                             call=call, mesh=mesh)
        _RUNNER = call
    return _RUNNER


def run(x, Wq, Wk, Wv, Wo, bo, trace=False):
    from concourse._compat import axon_active
    in_maps = make_in_maps(x, Wq, Wk, Wv, Wo)
    if axon_active():
        summed = _get_runner()(in_maps)
        acc = summed["outT"].astype(np.float64)
        results = summed
    else:
        # native /dev/neuron* path (non-axon environments)
        from concourse import bass_utils
        r = bass_utils.run_bass_kernel_spmd(
            _get_compiled(), in_maps, core_ids=list(range(NCORES)), trace=trace)
        results = r.results
        acc = np.zeros((B, D, S), dtype=np.float64)
        for c in range(NCORES):
            acc += np.asarray(results[c]["outT"], dtype=np.float64)
    out = acc.transpose(0, 2, 1) + np.asarray(bo, dtype=np.float64)
    return out.astype(np.float32), results


def kernel(x, Wq, Wk, Wv, Wo, bo):
    out, _ = run(np.asarray(x), np.asarray(Wq), np.asarray(Wk),
                 np.asarray(Wv), np.asarray(Wo), np.asarray(bo))
    return out


# revision 54
# speedup vs baseline: 1.4168x; 1.0519x over previous
"""Multi-head attention (B=2, S=2048, D=1024, H=16, no mask) on 8 TRN2 cores.

Sharding: tensor-parallel over heads — 2 heads per core. Each core computes
its heads' QKV projections, attention, and a partial out-projection
(row-sharded Wo); the host sums the 8 partials and adds the bias (the
all-reduce happens at gather time).

Device layout (per core), v3 — bf16 dataflow, software-pipelined attention,
transposed PV with per-partition softmax normalization:
  - All tensor data bf16 (x, W, q/k/v, exp-scores, ctx, partial out);
    matmul accumulation and softmax statistics stay fp32 in PSUM.
  - qT/kT kept transposed (feat-on-partitions); v projected DIRECTLY in
    token-major layout ([tokens, head, HD+1] va tiles, ones column
    prebaked) via per-token-tile matmuls — no PE transpose pass for v.
  - scoresT[k, q] = k @ qT per (head, key-tile): the two heads' K=64
    matmuls sit on array row-groups 0/64 and run concurrently on HW.
  - The attention inner loop is software-pipelined: QK(sk+1) is emitted
    BEFORE PV(sk), so the in-order PE never parks the next score matmul
    behind a PV that waits on exp(sk); ACT (the exp engine, the largest
    single engine load) stays saturated.
  - PV is TRANSPOSED: lhsT = exp-scores slice [keys 128, q-tile 128]
    (stationary, full 128-wide array), rhs = va [keys, HD+1] -> psum
    ctx[q, HD+1]; the va ones column puts the softmax denominator in psum
    COLUMN 64, i.e. a per-partition scalar.
  - Normalize: DVE reciprocal on the [128, 1] denominator column +
    per-partition tensor_scalar multiply (no cross-partition scatter
    DMAs, no PE broadcast matmuls), then PE transposes per (head,
    q-tile) put ctx back feature-major for the out-projection. Transpose
    outputs must start at psum partition 0, so head 1's rows reach ctxT
    partitions 64..127 via an SBUF-SBUF DMA — except in the drain tail,
    where the out-projection instead runs as two K=64 accumulating
    matmuls (head 1 read from its partition-0 staging tile) to keep that
    DMA off the critical path.
  - Cross-phase overlap via two 'pending unit' queues (PE-light normalize
    tails vs PE-heavy projection/out-projection blocks), drained one of
    each per sk-step into the ACT-paced attention loop's PE bubbles; the
    next query block's first QK is pre-emitted inside the current block's
    last step so exp never waits at block boundaries.
"""
import numpy as np

B = 2
S = 2048
D = 1024
H = 16
HD = 64
NCORES = 8
HPC = H // NCORES       # heads per core
FPC = HPC * HD          # 128 features per core


def build_mha_kernel(tc, outT, xT, wqT, wkT, wvT, woT, s=S, d=D):
    """Emit the per-core MHA program.

    outT: [B, d, s] bf16 (partial output, transposed, per-batch)
    xT:   [B, d, s] bf16
    wqT/wkT/wvT: [128, d//128, FPC] bf16, host pre-arranged so the
        weight DMA is contiguous (wqT pre-scaled by 1/sqrt(HD))
    woT:  [FPC, d] bf16
    """
    import concourse.mybir as mybir
    from contextlib import ExitStack

    nc = tc.nc
    f32 = mybir.dt.float32
    f32r = mybir.dt.float32r
    bf16 = mybir.dt.bfloat16
    Exp = mybir.ActivationFunctionType.Exp

    KT = d // 128           # contraction tiles for projections
    SK = s // 128           # key tiles per batch
    NCH = s // 512          # 512-token chunks
    NQB = s // 512          # query blocks
    QTPB = 4                # 128-wide q-tiles per 512-wide query block

    with ExitStack() as es:
        consts = es.enter_context(tc.tile_pool(name="consts", bufs=1))
        wpool = es.enter_context(tc.tile_pool(name="w", bufs=1))
        xpool = es.enter_context(tc.tile_pool(name="xt", bufs=2))
        qkv = es.enter_context(tc.tile_pool(name="qkv", bufs=2))
        vapool = es.enter_context(tc.tile_pool(name="va", bufs=1))
        epool = es.enter_context(tc.tile_pool(name="exp", bufs=2))
        cpool = es.enter_context(tc.tile_pool(name="ctxT", bufs=2))
        spool = es.enter_context(tc.tile_pool(name="small", bufs=2))
        opool = es.enter_context(tc.tile_pool(name="o", bufs=4))
        ps_sc = es.enter_context(tc.tile_pool(name="pssc", bufs=2, space="PSUM"))
        ps_ctx = es.enter_context(tc.tile_pool(name="psctx", bufs=1, space="PSUM"))
        ps_wk = es.enter_context(tc.tile_pool(name="pswk", bufs=2, space="PSUM"))

        from concourse.masks import make_identity
        identity = consts.tile([128, 128], bf16, tag="ident")
        make_identity(nc, identity[:])

        # --- weights (resident whole kernel). wq+wk on the SP queue (needed
        # first); wv/wo on the Pool queue. The ACT queue is kept clear of
        # DMA dispatches so exp issue is never delayed.
        wq_sb = wpool.tile([128, KT, FPC], bf16, tag="wq")
        nc.sync.dma_start(wq_sb[:], wqT)
        wk_sb = wpool.tile([128, KT, FPC], bf16, tag="wk")
        nc.sync.dma_start(wk_sb[:], wkT)
        wv_sb = wpool.tile([128, KT, FPC], bf16, tag="wv")
        nc.gpsimd.dma_start(wv_sb[:], wvT)
        wo_sb = wpool.tile([128, d], bf16, tag="wo")
        nc.gpsimd.dma_start(wo_sb[:], woT)

        # --- va tiles: [tokens 128, head, HD+1] with a persistent ones
        # column at [:, :, HD] (written once; v columns rewritten per batch)
        vas = []
        for sk in range(SK):
            va = vapool.tile([128, HPC, HD + 1], bf16, tag=f"va{sk}")
            nc.gpsimd.memset(va[:, :, HD:HD + 1], 1.0)
            vas.append(va)

        # --- x loads. batch 0: per-(tile, chunk) pieces, chunk-major, so the
        # first k-proj chunk can start after ~1/4 of the data; batch 1: whole
        # tiles. Split across the SP and Pool HWDGE queues.
        xts = {}
        for b in range(B):
            for k in range(KT):
                xts[(b, k)] = xpool.tile([128, s], bf16, tag=f"x{k}",
                                         name=f"x{b}_{k}")
        for ch in range(NCH):
            for k in range(KT):
                eng = nc.gpsimd if (ch * KT + k) % 2 else nc.sync
                cs = slice(ch * 512, (ch + 1) * 512)
                eng.dma_start(xts[(0, k)][:, cs], xT[0, k * 128:(k + 1) * 128, cs])
        for k in range(KT):
            eng = nc.gpsimd if k % 2 else nc.sync
            eng.dma_start(xts[(1, k)][:], xT[1, k * 128:(k + 1) * 128, :])

        # ---------- unit builders (each emits a small instruction group) ----
        def proj_chunk(dst, w_sb, b, ch):
            # feat-major projection chunk: dst[:, ch*512:+512] (for q/k)
            cs = slice(ch * 512, (ch + 1) * 512)
            pt = ps_wk.tile([128, 512], f32, tag="wk")
            for k in range(KT):
                nc.tensor.matmul(pt[:], w_sb[:, k, :], xts[(b, k)][:, cs],
                                 start=(k == 0), stop=(k == KT - 1))
            nc.vector.tensor_copy(dst[:, cs], pt[:])

        def proj_chunk_halves(dst, w_sb, b, ch):
            # proj_chunk split into two pending units (halves the PE burst a
            # unit injects into the ACT-paced loop). The psum ring has 2
            # slots and at most one other unit runs between the halves, so
            # the accumulator survives; the two halves MUST stay adjacent
            # in the heavies queue.
            cs = slice(ch * 512, (ch + 1) * 512)
            state = {}

            def half1():
                pt = ps_wk.tile([128, 512], f32, tag="wk", name="pt")
                for k in range(KT // 2):
                    nc.tensor.matmul(pt[:], w_sb[:, k, :], xts[(b, k)][:, cs],
                                     start=(k == 0), stop=False)
                state["pt"] = pt

            def half2():
                pt = state.pop("pt")
                for k in range(KT // 2, KT):
                    nc.tensor.matmul(pt[:], w_sb[:, k, :], xts[(b, k)][:, cs],
                                     start=False, stop=(k == KT - 1))
                nc.vector.tensor_copy(dst[:, cs], pt[:])

            return [half1, half2]

        def vproj_unit(b, sk):
            # token-major v projection: va[sk] tokens sk*128..+128, both heads
            ts_ = slice(sk * 128, (sk + 1) * 128)
            vp = ps_wk.tile([128, 512], f32, tag="wk")
            for k in range(KT):
                nc.tensor.matmul(vp[:, 0:FPC], xts[(b, k)][:, ts_],
                                 wv_sb[:, k, :],
                                 start=(k == 0), stop=(k == KT - 1))
            src = vp[:, 0:FPC].rearrange("p (j f) -> p j f", j=HPC)
            nc.vector.tensor_copy(vas[sk][:, :, 0:HD], src)

        def emit_outproj(ctxT_b, bb, m, ch, eng, split=False):
            # one partial out-projection unit: outT[bb] tile (m, ch)
            ms = slice(m * 128, (m + 1) * 128)
            cs = slice(ch * 512, (ch + 1) * 512)
            op = ps_wk.tile([128, 512], f32, tag="wk")
            if split:
                # tail variant: head 1's context read from its SBUF staging
                # tile (partitions 0..63) via a second K=64 matmul
                nc.tensor.matmul(op[:], wo_sb[0:HD, ms], ctxT_b[0:HD, cs],
                                 start=True, stop=False)
                nc.tensor.matmul(op[:], wo_hi[:, ms], cn2_stash[ch][:],
                                 start=False, stop=True)
            else:
                nc.tensor.matmul(op[:], wo_sb[:, ms], ctxT_b[:, cs],
                                 start=True, stop=True)
            ot = opool.tile([128, 512], bf16, tag="ot")
            if eng is nc.scalar:
                eng.copy(ot[:], op[:])
            else:
                eng.tensor_copy(ot[:], op[:])
            nc.sync.dma_start(outT[bb, ms, cs], ot[:])

        # pending unit queues, drained into the attention loop's PE bubbles:
        # `lights` are PE-light normalize tails, `heavies` are PE-heavy
        # projection / out-projection blocks. One of each per sk-step.
        lights = []
        heavies = []

        def drain(n_heavy=1):
            if lights:
                lights.pop(0)()
            popped = 0
            while heavies and popped < n_heavy:
                heavies.pop(0)()
                popped += 1
            if not popped and lights:
                lights.pop(0)()

        # q/k tiles per batch: allocate both batches' ring slots up front so
        # units queued during batch 0 write the buffers batch 1 will read.
        qTs = [qkv.tile([128, s], bf16, tag="q", name=f"qT{b}")
               for b in range(B)]
        kTs = [qkv.tile([128, s], bf16, tag="k", name=f"kT{b}")
               for b in range(B)]
        ctxTs = [cpool.tile([128, s], bf16, tag="ctxT", name=f"ctxT{b}")
                 for b in range(B)]

        # second copy of wo's high rows at base partition 0: lets the drain
        # tail's out-projections take head 1's context from SBUF directly
        # (two K=64 accumulating matmuls) instead of waiting on the
        # cross-partition DMA into ctxT
        wo_hi = wpool.tile([HD, d], bf16, tag="wo_hi")
        nc.vector.tensor_copy(wo_hi[:], wo_sb[HD:2 * HD, :])
        cn2_stash = {}

        def norm_unit(h, cs_h, ctxT, qb, release, skip_dma=False):
            # per (head, qb): 1/denom columns for all 4 q-tiles in one DVE
            # reciprocal, per-partition multiplies on Pool (SBUF-only), PE
            # transposes back to feature-major into ONE psum tile (head 1
            # lands on partitions 64..127 via col tile_position), single
            # DVE evacuation into ctxT.
            rc = spool.tile([128, QTPB], f32, tag=f"rc{h}", name="rc")
            nc.vector.reciprocal(rc[:], cs_h[:, :, HD])
            cn = spool.tile([128, QTPB, HD], bf16, tag=f"cn{h}", name="cn")
            with nc.allow_low_precision(reason="ctx rounds to bf16 anyway"):
                for qt in range(QTPB):
                    nc.gpsimd.tensor_scalar_mul(cn[:, qt, :],
                                                cs_h[:, qt, 0:HD],
                                                rc[:, qt:qt + 1])
            # bf16 transposes at 1 cycle/row (vs 2 for f32) into a packed
            # bf16 view of an f32 psum work tile
            wkt = ps_wk.tile([128, 512], f32, tag="wk", name="tpw")
            tpv = wkt[0:HD, 0:256].bitcast(bf16)
            for qt in range(QTPB):
                nc.tensor.transpose(tpv[:, qt * 128:(qt + 1) * 128],
                                    cn[:, qt, :], identity[:])
            if h == 0:
                nc.vector.tensor_copy(ctxT[0:HD, qb * 512:(qb + 1) * 512],
                                      tpv)
            else:
                # transpose outputs must start at psum partition 0; head 1's
                # rows reach ctxT partitions 64..127 via an SBUF-SBUF DMA
                cn2 = spool.tile([HD, 512], bf16, tag="cn2", name="cn2")
                nc.vector.tensor_copy(cn2[:], tpv)
                if skip_dma:
                    cn2_stash[qb] = cn2
                else:
                    nc.gpsimd.dma_start(
                        ctxT[HD:2 * HD, qb * 512:(qb + 1) * 512], cn2[:])
            if release:
                # ctxT columns for this qb are complete: release the
                # out-projection units for it
                heavies.extend(release)

        # batch 0 projections: attention can start once k-chunk0 (keys 0-511
        # = sk 0-3) and q-chunk0 are in; later k chunks are woven ahead of
        # the v-proj units into qb0's early steps (chunk c is consumed from
        # sk=4c, drained at step c-1). Steps 0..NCH-2 drain two heavies so
        # the v-proj units stay on their just-in-time schedule.
        proj_chunk(kTs[0], wk_sb, 0, 0)
        proj_chunk(qTs[0], wq_sb, 0, 0)
        weave = [(lambda ch=ch: proj_chunk(kTs[0], wk_sb, 0, ch))
                 for ch in range(1, NCH)]
        vunits = [(lambda sk=sk: vproj_unit(0, sk)) for sk in range(SK)]
        for i, u in enumerate(weave):
            vunits.insert(2 * i + 1, u)
        heavies.extend(vunits)

        # ---------- attention ----------
        b1proj_left = [0]
        qk_pre = {}
        for b in range(B):
            qT, kT, ctxT = qTs[b], kTs[b], ctxTs[b]
            for qb in range(NQB):
                qs = slice(qb * 512, (qb + 1) * 512)

                # ctx accumulators: per head, 4 q-tiles x (HD+1) packed in
                # one PSUM bank ([128, 4, 128] fp32, slices [:, qt, 0:65])
                accs = [ps_ctx.tile([128, QTPB, 128], f32, tag=f"acc{h}",
                                    name=f"acc{h}")
                        for h in range(HPC)]
                ets = {}

                def emit_qk(sk, qT=qT, kT=kT, qs=qs, ets=ets, store=None):
                    sps = ps_sc.tile([128, 2 * 512], f32, tag="sc", name="sps")
                    for h in range(HPC):
                        hr = slice(h * HD, (h + 1) * HD)
                        nc.tensor.matmul(
                            sps[:, h * 512:(h + 1) * 512],
                            kT[hr, sk * 128:(sk + 1) * 128],
                            qT[hr, qs], start=True, stop=True)
                    et = epool.tile([128, 2 * 512], bf16, tag=f"et{sk % 2}",
                                    name="et")
                    nc.scalar.activation(et[:], sps[:], Exp)
                    if store is not None:
                        qk_pre[store] = et
                    else:
                        ets[sk] = et

                def emit_pv(sk, accs=accs, ets=ets):
                    # transposed PV: exp-scores stationary (full width),
                    # va moving; ctx[q, hd] + denominator column in psum
                    et = ets.pop(sk)
                    for h in range(HPC):
                        for qt in range(QTPB):
                            # the 4 q-tile accumulators share one psum bank
                            # (= one 2KB zero region): only the bank's FIRST
                            # matmul starts (zeroing the whole region), only
                            # its LAST stops
                            nc.tensor.matmul(
                                accs[h][:, qt, 0:HD + 1],
                                et[:, h * 512 + qt * 128:
                                   h * 512 + (qt + 1) * 128],
                                vas[sk][:, h, :],
                                start=(sk == 0 and qt == 0),
                                stop=(sk == SK - 1 and qt == QTPB - 1))

                # software-pipelined sk loop: QK(sk+1) lands before PV(sk),
                # pending units fill the gap where PV waits on exp. The
                # FIRST QK of this qb may have been pre-emitted in the
                # previous qb's last step (qk_pre holds its exp tile).
                if qb in qk_pre:
                    ets[0] = qk_pre.pop(qb)
                else:
                    emit_qk(0)
                if b == 0 and qb + 1 < NQB:
                    # next q chunk, as two PE-light front-of-queue units
                    # (not needed until the next qb; emitting directly here
                    # would stall exp behind 1.7us of projection)
                    qh = proj_chunk_halves(qT, wq_sb, 0, qb + 1)
                    lights.insert(0, qh[0])
                    lights.insert(1, qh[1])
                # PV trails QK by TWO steps: PV(sk) lands at step sk+1, so
                # the first PV of a qb is emitted one step into its loop and
                # never stalls on the previous qb's accumulator evacuation.
                for sk in range(SK):
                    if sk + 1 < SK:
                        emit_qk(sk + 1)
                    elif qb + 1 < NQB:
                        # pre-emit the next qb's first QK so its exp starts
                        # right after this qb's last one
                        emit_qk(0, qs=slice((qb + 1) * 512, (qb + 2) * 512),
                                store=(qb + 1))
                    drain(2 if (b == 0 and qb == 0 and sk % 2 == 1
                                and sk < 2 * (NCH - 1)) else 1)
                    if sk >= 1:
                        emit_pv(sk - 1)
                emit_pv(SK - 1)

                # evacuate the raw accumulators to SBUF right away (frees
                # the psum banks for the next qb); normalize tails become
                # PE-light pending units.
                css = []
                for h in range(HPC):
                    cs_h = spool.tile([128, QTPB, HD + 1], f32, tag=f"cs{h}",
                                      name=f"cs{h}")
                    nc.vector.tensor_copy(cs_h[:], accs[h][:, :, 0:HD + 1])
                    css.append(cs_h)

                # out-projection units for this qb, released by the last
                # normalize unit (they read the ctxT columns it completes).
                # The final qb's units run in the drain tail where ACT is
                # idle: alternate their psum evacuations DVE/ACT there.
                tail_qb = (b == B - 1 and qb == NQB - 1)
                opr = [(lambda c=ctxT, bb=b, mm=m, cc=qb, sp=tail_qb,
                        e=(nc.scalar if (tail_qb and m % 2) else nc.vector):
                        emit_outproj(c, bb, mm, cc, e, split=sp))
                       for m in range(KT)]
                # head 1 first: its ctxT rows travel by SBUF-SBUF DMA, so
                # putting it ahead lets that latency overlap head 0's work
                for h in reversed(range(HPC)):
                    lights.append(
                        lambda h=h, cs_h=css[h], ctxT=ctxT, qb=qb,
                        release=(opr if h == 0 else None), sd=tail_qb:
                            norm_unit(h, cs_h, ctxT, qb, release,
                                      skip_dma=(sd and h == 1)))

                if b == 0 and qb == min(1, NQB - 2):
                    # batch 1's k projection + all q chunks, into the tail
                    # of batch 0's attention (the qkv ring slots for batch 1
                    # are free; x(1) tiles have loaded long since)
                    def done(u):
                        def f():
                            u()
                            b1proj_left[0] -= 1
                        return f
                    units = []
                    for ch in range(NCH):
                        units += proj_chunk_halves(kTs[1], wk_sb, 1, ch)
                    for ch in range(NCH):
                        units += proj_chunk_halves(qTs[1], wq_sb, 1, ch)
                    b1proj_left[0] = len(units)
                    heavies.extend(done(u) for u in units)

            if b == 0:
                # make sure batch 1's q/k are in place before its attention
                # emits reads of them (no-op at full size: they drained
                # into qb2/qb3's bubbles already)
                while b1proj_left[0] > 0:
                    drain()
                # batch 1's v tiles refill inside batch 1's qb0 bubbles,
                # ahead of everything else queued (its PV needs va[sk] by
                # step sk)
                heavies[0:0] = [
                    (lambda sk=sk: vproj_unit(1, sk)) for sk in range(SK)]

        # drain everything left (batch-1 norm tails + outproj backlog)
        while lights or heavies:
            drain()


_CACHE = {}


def _get_compiled(s=S, d=D, reps=1):
    key = (s, d, reps)
    if key not in _CACHE:
        import concourse.bacc as bacc
        import concourse.tile as tile
        import concourse.mybir as mybir

        bf16 = mybir.dt.bfloat16
        nc = bacc.Bacc("TRN2", target_bir_lowering=False, debug=False)
        xT = nc.dram_tensor("xT", [B, d, s], bf16, kind="ExternalInput")
        wqT = nc.dram_tensor("wqT", [128, d // 128, FPC], bf16,
                             kind="ExternalInput")
        wkT = nc.dram_tensor("wkT", [128, d // 128, FPC], bf16,
                             kind="ExternalInput")
        wvT = nc.dram_tensor("wvT", [128, d // 128, FPC], bf16,
                             kind="ExternalInput")
        woT = nc.dram_tensor("woT", [FPC, d], bf16, kind="ExternalInput")
        outT = nc.dram_tensor("outT", [B, d, s], bf16, kind="ExternalOutput")
        with tile.TileContext(nc) as tc:
            for _ in range(reps):
                build_mha_kernel(tc, outT.ap(), xT.ap(), wqT.ap(), wkT.ap(),
                                 wvT.ap(), woT.ap(), s=s, d=d)
        nc.compile()
        _CACHE[key] = nc
    return _CACHE[key]


def _bf16(a):
    import ml_dtypes
    return np.ascontiguousarray(np.asarray(a, dtype=np.float32)).astype(
        ml_dtypes.bfloat16)


def make_in_maps(x, Wq, Wk, Wv, Wo):
    """Host-side shard prep: transpose x, slice + transpose weights per core."""
    b, s, d = x.shape
    xT = _bf16(x.transpose(0, 2, 1))
    scale = np.float32(1.0 / np.sqrt(HD))

    def prearr(wt):
        # [d, FPC] -> [128, d//128, FPC] so each SBUF partition row is one
        # contiguous DMA line (avoids 512B-descriptor strided reads)
        return _bf16(wt.reshape(d // 128, 128, FPC).transpose(1, 0, 2))

    in_maps = []
    for c in range(NCORES):
        if (c + 1) * FPC > d:
            # small-D sim configs: fewer head-slices than cores
            in_maps.append(in_maps[0])
            continue
        rs = slice(c * FPC, (c + 1) * FPC)
        in_maps.append({
            "xT": xT,
            "wqT": prearr((Wq[rs, :] * scale).T.astype(np.float32)),
            "wkT": prearr(Wk[rs, :].T.astype(np.float32)),
            "wvT": prearr(Wv[rs, :].T.astype(np.float32)),
            "woT": _bf16(Wo[:, rs].T),
        })
    return in_maps


_RUNNER = None
_RUNNER_STATE = {}


def _get_runner():
    """Build (once) a cached jitted SPMD executor mirroring
    bass2jax.run_bass_via_pjrt's multi-core path."""
    global _RUNNER
    if _RUNNER is None:
        import jax
        import jax.numpy as jnp
        from jax.sharding import Mesh, PartitionSpec
        from jax.experimental.shard_map import shard_map
        import concourse.mybir as mybir
        from concourse import bass2jax

        nc = _get_compiled()
        bass2jax.install_neuronx_cc_hook()

        partition_name = (nc.partition_id_tensor.name
                          if nc.partition_id_tensor else None)
        in_names = []
        out_names = []
        out_avals = []
        for alloc in nc.m.functions[0].allocations:
            if not isinstance(alloc, mybir.MemoryLocationSet):
                continue
            name = alloc.memorylocations[0].name
            if alloc.kind == "ExternalInput":
                if name != partition_name:
                    in_names.append(name)
            elif alloc.kind == "ExternalOutput":
                out_names.append(name)
                out_avals.append(jax.core.ShapedArray(
                    tuple(alloc.tensor_shape), mybir.dt.np(alloc.dtype)))
        n_outs = len(out_names)
        all_names = in_names + out_names
        if partition_name is not None:
            all_names = all_names + [partition_name]

        def _body(*args):
            operands = list(args)
            if partition_name is not None:
                operands.append(bass2jax.partition_id_tensor())
            outs = bass2jax._bass_exec_p.bind(
                *operands,
                out_avals=tuple(out_avals),
                in_names=tuple(all_names),
                out_names=tuple(out_names),
                lowering_input_output_aliases=(),
                sim_require_finite=True,
                sim_require_nnan=True,
                nc=nc,
            )
            return tuple(outs)

        devices = jax.devices()[:NCORES]
        mesh = Mesh(np.asarray(devices), ("core",))
        # xT is identical on every core: replicate it instead of concatenating
        # 8 copies on the host.
        in_specs = tuple(PartitionSpec() if name == "xT" else PartitionSpec("core")
                         for name in in_names)
        sharded = jax.jit(
            shard_map(_body, mesh=mesh,
                      in_specs=in_specs + (PartitionSpec("core"),) * n_outs,
                      out_specs=(PartitionSpec("core"),) * n_outs,
                      check_rep=False),
            keep_unused=True)

        # separate jit: on-device sum of the 8 per-core partials (all-reduce)
        def _reduce(a):
            return jnp.sum(a.reshape((NCORES,) + tuple(out_avals[0].shape))
                           .astype(jnp.float32), axis=0)
        reduce_jit = jax.jit(_reduce)

        out_shapes = [tuple(a.shape) for a in out_avals]
        out_dtypes = [a.dtype for a in out_avals]
        zeros_dev = [None]

        from jax.sharding import NamedSharding
        rep_shd = NamedSharding(mesh, PartitionSpec())

        def call(in_maps):
            args = []
            for name in in_names:
                if name == "xT":
                    # one host->device transfer, then device-side broadcast
                    xd = jax.device_put(np.asarray(in_maps[0][name]),
                                        devices[0])
                    args.append(jax.device_put(xd, rep_shd))
                else:
                    args.append(np.concatenate(
                        [np.asarray(m[name]) for m in in_maps], axis=0))
            if zeros_dev[0] is None:
                from jax.sharding import NamedSharding
                shd = NamedSharding(mesh, PartitionSpec("core"))
                zeros_dev[0] = [
                    jax.device_put(
                        np.zeros((NCORES * sh[0],) + sh[1:], dt), shd)
                    for sh, dt in zip(out_shapes, out_dtypes)]
            outs = sharded(*args, *zeros_dev[0])
            try:
                summed = np.asarray(reduce_jit(outs[0]))
            except Exception:
                # device reduce unavailable: fetch partials, sum on host
                a = np.asarray(outs[0]).astype(np.float64)
                summed = a.reshape((NCORES,) + tuple(out_avals[0].shape)).sum(0)
            return {out_names[0]: summed}

        _RUNNER_STATE.update(sharded=sharded, in_names=in_names,
                             out_shapes=out_shapes, out_dtypes=out_dtypes,
                             call=call, mesh=mesh)
        _RUNNER = call
    return _RUNNER


def run(x, Wq, Wk, Wv, Wo, bo, trace=False):
    from concourse._compat import axon_active
    in_maps = make_in_maps(x, Wq, Wk, Wv, Wo)
    if axon_active():
        summed = _get_runner()(in_maps)
        acc = summed["outT"].astype(np.float64)
        results = summed
    else:
        # native /dev/neuron* path (non-axon environments)
        from concourse import bass_utils
        r = bass_utils.run_bass_kernel_spmd(
            _get_compiled(), in_maps, core_ids=list(range(NCORES)), trace=trace)
        results = r.results
        acc = np.zeros((B, D, S), dtype=np.float64)
        for c in range(NCORES):
            acc += np.asarray(results[c]["outT"], dtype=np.float64)
    out = acc.transpose(0, 2, 1) + np.asarray(bo, dtype=np.float64)
    return out.astype(np.float32), results


def kernel(x, Wq, Wk, Wv, Wo, bo):
    out, _ = run(np.asarray(x), np.asarray(Wq), np.asarray(Wk),
                 np.asarray(Wv), np.asarray(Wo), np.asarray(bo))
    return out


# revision 64
# speedup vs baseline: 1.4255x; 1.0061x over previous
"""Multi-head attention (B=2, S=2048, D=1024, H=16, no mask) on 8 TRN2 cores.

Sharding: tensor-parallel over heads — 2 heads per core. Each core computes
its heads' QKV projections, attention, and a partial out-projection
(row-sharded Wo); the host sums the 8 partials and adds the bias (the
all-reduce happens at gather time).

Device layout (per core), v3 — bf16 dataflow, software-pipelined attention,
transposed PV with per-partition softmax normalization:
  - All tensor data bf16 (x, W, q/k/v, exp-scores, ctx, partial out);
    matmul accumulation and softmax statistics stay fp32 in PSUM.
  - qT/kT kept transposed (feat-on-partitions); v projected DIRECTLY in
    token-major layout ([tokens, head, HD+1] va tiles, ones column
    prebaked) via per-token-tile matmuls — no PE transpose pass for v.
  - scoresT[k, q] = k @ qT per (head, key-tile): the two heads' K=64
    matmuls sit on array row-groups 0/64 and run concurrently on HW.
  - The attention inner loop is software-pipelined: QK(sk+1) is emitted
    BEFORE PV(sk), so the in-order PE never parks the next score matmul
    behind a PV that waits on exp(sk); ACT (the exp engine, the largest
    single engine load) stays saturated.
  - PV is TRANSPOSED: lhsT = exp-scores slice [keys 128, q-tile 128]
    (stationary, full 128-wide array), rhs = va [keys, HD+1] -> psum
    ctx[q, HD+1]; the va ones column puts the softmax denominator in psum
    COLUMN 64, i.e. a per-partition scalar.
  - Normalize: DVE reciprocal on the [128, 1] denominator column +
    per-partition tensor_scalar multiply (no cross-partition scatter
    DMAs, no PE broadcast matmuls), then PE transposes per (head,
    q-tile) put ctx back feature-major for the out-projection. Transpose
    outputs must start at psum partition 0, so head 1's rows reach ctxT
    partitions 64..127 via an SBUF-SBUF DMA — except in the drain tail,
    where the out-projection instead runs as two K=64 accumulating
    matmuls (head 1 read from its partition-0 staging tile) to keep that
    DMA off the critical path.
  - Cross-phase overlap via two 'pending unit' queues (PE-light normalize
    tails vs PE-heavy projection/out-projection blocks), drained one of
    each per sk-step into the ACT-paced attention loop's PE bubbles; the
    next query block's first QK is pre-emitted inside the current block's
    last step so exp never waits at block boundaries.
"""
import numpy as np

B = 2
S = 2048
D = 1024
H = 16
HD = 64
NCORES = 8
HPC = H // NCORES       # heads per core
FPC = HPC * HD          # 128 features per core


def build_mha_kernel(tc, outT, xT, wqT, wkT, wvT, woT, s=S, d=D):
    """Emit the per-core MHA program.

    outT: [B, d, s] bf16 (partial output, transposed, per-batch)
    xT:   [B, d, s] bf16
    wqT/wkT/wvT: [128, d//128, FPC] bf16, host pre-arranged so the
        weight DMA is contiguous (wqT pre-scaled by 1/sqrt(HD))
    woT:  [FPC, d] bf16
    """
    import concourse.mybir as mybir
    from contextlib import ExitStack

    nc = tc.nc
    f32 = mybir.dt.float32
    f32r = mybir.dt.float32r
    bf16 = mybir.dt.bfloat16
    Exp = mybir.ActivationFunctionType.Exp

    KT = d // 128           # contraction tiles for projections
    SK = s // 128           # key tiles per batch
    NCH = s // 512          # 512-token chunks
    NQB = s // 512          # query blocks
    QTPB = 4                # 128-wide q-tiles per 512-wide query block

    with ExitStack() as es:
        consts = es.enter_context(tc.tile_pool(name="consts", bufs=1))
        wpool = es.enter_context(tc.tile_pool(name="w", bufs=1))
        xpool = es.enter_context(tc.tile_pool(name="xt", bufs=2))
        qkv = es.enter_context(tc.tile_pool(name="qkv", bufs=2))
        vapool = es.enter_context(tc.tile_pool(name="va", bufs=1))
        epool = es.enter_context(tc.tile_pool(name="exp", bufs=2))
        cpool = es.enter_context(tc.tile_pool(name="ctxT", bufs=2))
        spool = es.enter_context(tc.tile_pool(name="small", bufs=2))
        opool = es.enter_context(tc.tile_pool(name="o", bufs=4))
        ps_sc = es.enter_context(tc.tile_pool(name="pssc", bufs=2, space="PSUM"))
        ps_ctx = es.enter_context(tc.tile_pool(name="psctx", bufs=1, space="PSUM"))
        ps_wk = es.enter_context(tc.tile_pool(name="pswk", bufs=2, space="PSUM"))

        from concourse.masks import make_identity
        identity = consts.tile([128, 128], bf16, tag="ident")
        make_identity(nc, identity[:])

        # --- weights (resident whole kernel). wq+wk on the SP queue (needed
        # first); wv/wo on the Pool queue. The ACT queue is kept clear of
        # DMA dispatches so exp issue is never delayed.
        wk_sb = wpool.tile([128, KT, FPC], bf16, tag="wk")
        nc.sync.dma_start(wk_sb[:], wkT)
        wq_sb = wpool.tile([128, KT, FPC], bf16, tag="wq")
        wv_sb = wpool.tile([128, KT, FPC], bf16, tag="wv")
        nc.gpsimd.dma_start(wv_sb[:], wvT)
        wo_sb = wpool.tile([128, d], bf16, tag="wo")
        nc.gpsimd.dma_start(wo_sb[:], woT)

        # --- va tiles: [tokens 128, head, HD+1] with a persistent ones
        # column at [:, :, HD] (written once; v columns rewritten per batch)
        vas = []
        for sk in range(SK):
            va = vapool.tile([128, HPC, HD + 1], bf16, tag=f"va{sk}")
            nc.gpsimd.memset(va[:, :, HD:HD + 1], 1.0)
            vas.append(va)

        # --- x loads. batch 0: per-(tile, chunk) pieces, chunk-major, so the
        # first k-proj chunk can start after ~1/4 of the data; batch 1: whole
        # tiles. Split across the SP and Pool HWDGE queues.
        xts = {}
        for b in range(B):
            for k in range(KT):
                xts[(b, k)] = xpool.tile([128, s], bf16, tag=f"x{k}",
                                         name=f"x{b}_{k}")
        for ch in range(NCH):
            for k in range(KT):
                eng = nc.gpsimd if (ch * KT + k) % 2 else nc.sync
                cs = slice(ch * 512, (ch + 1) * 512)
                eng.dma_start(xts[(0, k)][:, cs], xT[0, k * 128:(k + 1) * 128, cs])
            if ch == 0:
                # wq sits behind the chunk-0 x pieces: k-proj's critical
                # path is not delayed, and wq still lands before q0-proj
                nc.sync.dma_start(wq_sb[:], wqT)
        for k in range(KT):
            eng = nc.gpsimd if k % 2 else nc.sync
            eng.dma_start(xts[(1, k)][:], xT[1, k * 128:(k + 1) * 128, :])

        # ---------- unit builders (each emits a small instruction group) ----
        def proj_chunk(dst, w_sb, b, ch):
            # feat-major projection chunk: dst[:, ch*512:+512] (for q/k)
            cs = slice(ch * 512, (ch + 1) * 512)
            pt = ps_wk.tile([128, 512], f32, tag="wk")
            for k in range(KT):
                nc.tensor.matmul(pt[:], w_sb[:, k, :], xts[(b, k)][:, cs],
                                 start=(k == 0), stop=(k == KT - 1))
            nc.vector.tensor_copy(dst[:, cs], pt[:])

        def proj_chunk_halves(dst, w_sb, b, ch):
            # proj_chunk split into two pending units (halves the PE burst a
            # unit injects into the ACT-paced loop). The psum ring has 2
            # slots and at most one other unit runs between the halves, so
            # the accumulator survives; the two halves MUST stay adjacent
            # in the heavies queue.
            cs = slice(ch * 512, (ch + 1) * 512)
            state = {}

            def half1():
                pt = ps_wk.tile([128, 512], f32, tag="wk", name="pt")
                for k in range(KT // 2):
                    nc.tensor.matmul(pt[:], w_sb[:, k, :], xts[(b, k)][:, cs],
                                     start=(k == 0), stop=False)
                state["pt"] = pt

            def half2():
                pt = state.pop("pt")
                for k in range(KT // 2, KT):
                    nc.tensor.matmul(pt[:], w_sb[:, k, :], xts[(b, k)][:, cs],
                                     start=False, stop=(k == KT - 1))
                nc.vector.tensor_copy(dst[:, cs], pt[:])

            return [half1, half2]

        def vproj_unit(b, sk):
            # token-major v projection: va[sk] tokens sk*128..+128, both heads
            ts_ = slice(sk * 128, (sk + 1) * 128)
            vp = ps_wk.tile([128, 512], f32, tag="wk")
            for k in range(KT):
                nc.tensor.matmul(vp[:, 0:FPC], xts[(b, k)][:, ts_],
                                 wv_sb[:, k, :],
                                 start=(k == 0), stop=(k == KT - 1))
            src = vp[:, 0:FPC].rearrange("p (j f) -> p j f", j=HPC)
            nc.vector.tensor_copy(vas[sk][:, :, 0:HD], src)

        def emit_outproj(ctxT_b, bb, m, ch, eng, split=False):
            # one partial out-projection unit: outT[bb] tile (m, ch)
            ms = slice(m * 128, (m + 1) * 128)
            cs = slice(ch * 512, (ch + 1) * 512)
            op = ps_wk.tile([128, 512], f32, tag="wk")
            if split:
                # tail variant: head 1's context read from its SBUF staging
                # tile (partitions 0..63) via a second K=64 matmul
                nc.tensor.matmul(op[:], wo_sb[0:HD, ms], ctxT_b[0:HD, cs],
                                 start=True, stop=False)
                nc.tensor.matmul(op[:], wo_hi[:, ms], cn2_stash[ch][:],
                                 start=False, stop=True)
            else:
                nc.tensor.matmul(op[:], wo_sb[:, ms], ctxT_b[:, cs],
                                 start=True, stop=True)
            ot = opool.tile([128, 512], bf16, tag="ot")
            if eng is nc.scalar:
                eng.copy(ot[:], op[:])
            else:
                eng.tensor_copy(ot[:], op[:])
            nc.sync.dma_start(outT[bb, ms, cs], ot[:])

        # pending unit queues, drained into the attention loop's PE bubbles:
        # `lights` are PE-light normalize tails, `heavies` are PE-heavy
        # projection / out-projection blocks. One of each per sk-step.
        lights = []
        heavies = []

        def drain(n_heavy=1):
            if lights:
                lights.pop(0)()
            popped = 0
            while heavies and popped < n_heavy:
                heavies.pop(0)()
                popped += 1
            if not popped and lights:
                lights.pop(0)()

        # q/k tiles per batch: allocate both batches' ring slots up front so
        # units queued during batch 0 write the buffers batch 1 will read.
        qTs = [qkv.tile([128, s], bf16, tag="q", name=f"qT{b}")
               for b in range(B)]
        kTs = [qkv.tile([128, s], bf16, tag="k", name=f"kT{b}")
               for b in range(B)]
        ctxTs = [cpool.tile([128, s], bf16, tag="ctxT", name=f"ctxT{b}")
                 for b in range(B)]

        # second copy of wo's high rows at base partition 0: lets the drain
        # tail's out-projections take head 1's context from SBUF directly
        # (two K=64 accumulating matmuls) instead of waiting on the
        # cross-partition DMA into ctxT
        wo_hi = wpool.tile([HD, d], bf16, tag="wo_hi")
        nc.vector.tensor_copy(wo_hi[:], wo_sb[HD:2 * HD, :])
        cn2_stash = {}

        def norm_unit(h, cs_h, ctxT, qb, release, skip_dma=False, tail=False):
            # per (head, qb): 1/denom columns for all 4 q-tiles in one DVE
            # reciprocal, per-partition multiplies on Pool (SBUF-only), PE
            # transposes back to feature-major into ONE psum tile (head 1
            # lands on partitions 64..127 via col tile_position), single
            # DVE evacuation into ctxT.
            rc = spool.tile([128, QTPB], f32, tag=f"rc{h}", name="rc")
            nc.vector.reciprocal(rc[:], cs_h[:, :, HD])
            cn = spool.tile([128, QTPB, HD], bf16, tag=f"cn{h}", name="cn")
            # the drain tail has an idle DVE: run head 0's multiplies there
            # so the two heads' normalize chains overlap
            mul_eng = nc.vector if (tail and h == 0) else nc.gpsimd
            with nc.allow_low_precision(reason="ctx rounds to bf16 anyway"):
                for qt in range(QTPB):
                    mul_eng.tensor_scalar_mul(cn[:, qt, :],
                                              cs_h[:, qt, 0:HD],
                                              rc[:, qt:qt + 1])
            # bf16 transposes at 1 cycle/row (vs 2 for f32) into a packed
            # bf16 view of an f32 psum work tile
            wkt = ps_wk.tile([128, 512], f32, tag="wk", name="tpw")
            tpv = wkt[0:HD, 0:256].bitcast(bf16)
            for qt in range(QTPB):
                nc.tensor.transpose(tpv[:, qt * 128:(qt + 1) * 128],
                                    cn[:, qt, :], identity[:])
            if h == 0:
                nc.vector.tensor_copy(ctxT[0:HD, qb * 512:(qb + 1) * 512],
                                      tpv)
            else:
                # transpose outputs must start at psum partition 0; head 1's
                # rows reach ctxT partitions 64..127 via an SBUF-SBUF DMA
                cn2 = spool.tile([HD, 512], bf16, tag="cn2", name="cn2")
                nc.vector.tensor_copy(cn2[:], tpv)
                if skip_dma:
                    cn2_stash[qb] = cn2
                else:
                    nc.gpsimd.dma_start(
                        ctxT[HD:2 * HD, qb * 512:(qb + 1) * 512], cn2[:])
            if release:
                # ctxT columns for this qb are complete: release the
                # out-projection units for it
                heavies.extend(release)

        # batch 0 projections: attention can start once k-chunk0 (keys 0-511
        # = sk 0-3) and q-chunk0 are in; later k chunks are woven ahead of
        # the v-proj units into qb0's early steps (chunk c is consumed from
        # sk=4c, drained at step c-1). Steps 0..NCH-2 drain two heavies so
        # the v-proj units stay on their just-in-time schedule.
        proj_chunk(kTs[0], wk_sb, 0, 0)
        proj_chunk(qTs[0], wq_sb, 0, 0)
        kh = []
        for ch in range(1, NCH):
            kh += proj_chunk_halves(kTs[0], wk_sb, 0, ch)
        vs = [(lambda sk=sk: vproj_unit(0, sk)) for sk in range(SK)]
        if NCH == 4:
            # interleave the 6 k-halves so each v-proj unit still drains at
            # its just-in-time step (see drain(2) schedule below); each
            # half-pair stays 2 apart (one ring allocation between halves)
            order = [vs[0], kh[0], vs[1], kh[1], vs[2], vs[3], kh[2], vs[4],
                     kh[3], vs[5], vs[6], vs[7], kh[4], vs[8], kh[5], vs[9]]
            order += vs[10:]
        else:
            order = list(vs)
            for i, u in enumerate(kh):
                order.insert(2 * i + 1, u)
        heavies.extend(order)

        # ---------- attention ----------
        b1proj_left = [0]
        qk_pre = {}
        for b in range(B):
            qT, kT, ctxT = qTs[b], kTs[b], ctxTs[b]
            for qb in range(NQB):
                qs = slice(qb * 512, (qb + 1) * 512)

                # ctx accumulators: per head, 4 q-tiles x (HD+1) packed in
                # one PSUM bank ([128, 4, 128] fp32, slices [:, qt, 0:65])
                accs = [ps_ctx.tile([128, QTPB, 128], f32, tag=f"acc{h}",
                                    name=f"acc{h}")
                        for h in range(HPC)]
                ets = {}

                def emit_qk(sk, qT=qT, kT=kT, qs=qs, ets=ets, store=None):
                    sps = ps_sc.tile([128, 2 * 512], f32, tag="sc", name="sps")
                    for h in range(HPC):
                        hr = slice(h * HD, (h + 1) * HD)
                        nc.tensor.matmul(
                            sps[:, h * 512:(h + 1) * 512],
                            kT[hr, sk * 128:(sk + 1) * 128],
                            qT[hr, qs], start=True, stop=True)
                    et = epool.tile([128, 2 * 512], bf16, tag=f"et{sk % 2}",
                                    name="et")
                    nc.scalar.activation(et[:], sps[:], Exp)
                    if store is not None:
                        qk_pre[store] = et
                    else:
                        ets[sk] = et

                def emit_pv(sk, accs=accs, ets=ets):
                    # transposed PV: exp-scores stationary (full width),
                    # va moving; ctx[q, hd] + denominator column in psum
                    et = ets.pop(sk)
                    for h in range(HPC):
                        for qt in range(QTPB):
                            # the 4 q-tile accumulators share one psum bank
                            # (= one 2KB zero region): only the bank's FIRST
                            # matmul starts (zeroing the whole region), only
                            # its LAST stops
                            nc.tensor.matmul(
                                accs[h][:, qt, 0:HD + 1],
                                et[:, h * 512 + qt * 128:
                                   h * 512 + (qt + 1) * 128],
                                vas[sk][:, h, :],
                                start=(sk == 0 and qt == 0),
                                stop=(sk == SK - 1 and qt == QTPB - 1))

                # software-pipelined sk loop: QK(sk+1) lands before PV(sk),
                # pending units fill the gap where PV waits on exp. The
                # FIRST QK of this qb may have been pre-emitted in the
                # previous qb's last step (qk_pre holds its exp tile).
                if (b, qb) in qk_pre:
                    ets[0] = qk_pre.pop((b, qb))
                else:
                    emit_qk(0)
                if b == 0 and qb + 1 < NQB:
                    # next q chunk, as two PE-light front-of-queue units
                    # (not needed until the next qb; emitting directly here
                    # would stall exp behind 1.7us of projection)
                    qh = proj_chunk_halves(qT, wq_sb, 0, qb + 1)
                    lights.insert(0, qh[0])
                    lights.insert(1, qh[1])
                # PV trails QK by TWO steps: PV(sk) lands at step sk+1, so
                # the first PV of a qb is emitted one step into its loop and
                # never stalls on the previous qb's accumulator evacuation.
                # The final qb reverts to trail-1 so its last PV (and the
                # whole drain tail) starts one step earlier.
                trail = 1
                for sk in range(SK):
                    if sk + 1 < SK:
                        emit_qk(sk + 1)
                    elif qb + 1 < NQB:
                        # pre-emit the next qb's first QK so its exp starts
                        # right after this qb's last one
                        emit_qk(0, qs=slice((qb + 1) * 512, (qb + 2) * 512),
                                store=(b, qb + 1))
                    elif b == 0 and b1proj_left[0] == 0:
                        # batch seam: batch 1's q/k are ready (full-size
                        # schedule) - pre-emit its first QK too
                        emit_qk(0, qT=qTs[1], kT=kTs[1],
                                qs=slice(0, 512), store=(1, 0))
                    drain(2 if (b == 0 and qb == 0 and
                                sk in (1, 2, 4, 5, 8, 9)) else 1)
                    if sk >= trail:
                        emit_pv(sk - trail)
                if trail:
                    emit_pv(SK - 1)

                # evacuate the raw accumulators to SBUF right away (frees
                # the psum banks for the next qb); normalize tails become
                # PE-light pending units.
                css = []
                for h in range(HPC):
                    cs_h = spool.tile([128, QTPB, HD + 1], f32, tag=f"cs{h}",
                                      name=f"cs{h}")
                    nc.vector.tensor_copy(cs_h[:], accs[h][:, :, 0:HD + 1])
                    css.append(cs_h)

                # out-projection units for this qb, released by the last
                # normalize unit (they read the ctxT columns it completes).
                # The final qb's units run in the drain tail where ACT is
                # idle: alternate their psum evacuations DVE/ACT there.
                tail_qb = (b == B - 1 and qb == NQB - 1)
                opr = [(lambda c=ctxT, bb=b, mm=m, cc=qb, sp=tail_qb,
                        e=(nc.scalar if (tail_qb and m % 2) else nc.vector):
                        emit_outproj(c, bb, mm, cc, e, split=sp))
                       for m in range(KT)]
                # head 1 first: its ctxT rows travel by SBUF-SBUF DMA, so
                # putting it ahead lets that latency overlap head 0's work
                for h in reversed(range(HPC)):
                    lights.append(
                        lambda h=h, cs_h=css[h], ctxT=ctxT, qb=qb,
                        release=(opr if h == 0 else None), sd=tail_qb:
                            norm_unit(h, cs_h, ctxT, qb, release,
                                      skip_dma=(sd and h == 1), tail=sd))

                if b == 0 and qb == min(1, NQB - 2):
                    # batch 1's k projection + all q chunks, into the tail
                    # of batch 0's attention (the qkv ring slots for batch 1
                    # are free; x(1) tiles have loaded long since)
                    def done(u):
                        def f():
                            u()
                            b1proj_left[0] -= 1
                        return f
                    units = []
                    for ch in range(NCH):
                        units += proj_chunk_halves(kTs[1], wk_sb, 1, ch)
                    for ch in range(NCH):
                        units += proj_chunk_halves(qTs[1], wq_sb, 1, ch)
                    b1proj_left[0] = len(units)
                    heavies.extend(done(u) for u in units)

            if b == 0:
                # make sure batch 1's q/k are in place before its attention
                # emits reads of them (no-op at full size: they drained
                # into qb2/qb3's bubbles already)
                while b1proj_left[0] > 0:
                    drain()
                # batch 1's v tiles refill inside batch 1's qb0 bubbles,
                # ahead of everything else queued (its PV needs va[sk] by
                # step sk)
                heavies[0:0] = [
                    (lambda sk=sk: vproj_unit(1, sk)) for sk in range(SK)]

        # drain everything left (batch-1 norm tails + outproj backlog)
        while lights or heavies:
            drain()


_CACHE = {}


def _get_compiled(s=S, d=D, reps=1):
    key = (s, d, reps)
    if key not in _CACHE:
        import concourse.bacc as bacc
        import concourse.tile as tile
        import concourse.mybir as mybir

        bf16 = mybir.dt.bfloat16
        nc = bacc.Bacc("TRN2", target_bir_lowering=False, debug=False)
        xT = nc.dram_tensor("xT", [B, d, s], bf16, kind="ExternalInput")
        wqT = nc.dram_tensor("wqT", [128, d // 128, FPC], bf16,
                             kind="ExternalInput")
        wkT = nc.dram_tensor("wkT", [128, d // 128, FPC], bf16,
                             kind="ExternalInput")
        wvT = nc.dram_tensor("wvT", [128, d // 128, FPC], bf16,
                             kind="ExternalInput")
        woT = nc.dram_tensor("woT", [FPC, d], bf16, kind="ExternalInput")
        outT = nc.dram_tensor("outT", [B, d, s], bf16, kind="ExternalOutput")
        with tile.TileContext(nc) as tc:
            for _ in range(reps):
                build_mha_kernel(tc, outT.ap(), xT.ap(), wqT.ap(), wkT.ap(),
                                 wvT.ap(), woT.ap(), s=s, d=d)
        nc.compile()
        _CACHE[key] = nc
    return _CACHE[key]


def _bf16(a):
    import ml_dtypes
    return np.ascontiguousarray(np.asarray(a, dtype=np.float32)).astype(
        ml_dtypes.bfloat16)


def make_in_maps(x, Wq, Wk, Wv, Wo):
    """Host-side shard prep: transpose x, slice + transpose weights per core."""
    b, s, d = x.shape
    xT = _bf16(x.transpose(0, 2, 1))
    scale = np.float32(1.0 / np.sqrt(HD))

    def prearr(wt):
        # [d, FPC] -> [128, d//128, FPC] so each SBUF partition row is one
        # contiguous DMA line (avoids 512B-descriptor strided reads)
        return _bf16(wt.reshape(d // 128, 128, FPC).transpose(1, 0, 2))

    in_maps = []
    for c in range(NCORES):
        if (c + 1) * FPC > d:
            # small-D sim configs: fewer head-slices than cores
            in_maps.append(in_maps[0])
            continue
        rs = slice(c * FPC, (c + 1) * FPC)
        in_maps.append({
            "xT": xT,
            "wqT": prearr((Wq[rs, :] * scale).T.astype(np.float32)),
            "wkT": prearr(Wk[rs, :].T.astype(np.float32)),
            "wvT": prearr(Wv[rs, :].T.astype(np.float32)),
            "woT": _bf16(Wo[:, rs].T),
        })
    return in_maps


_RUNNER = None
_RUNNER_STATE = {}


def _get_runner():
    """Build (once) a cached jitted SPMD executor mirroring
    bass2jax.run_bass_via_pjrt's multi-core path."""
    global _RUNNER
    if _RUNNER is None:
        import jax
        import jax.numpy as jnp
        from jax.sharding import Mesh, PartitionSpec
        from jax.experimental.shard_map import shard_map
        import concourse.mybir as mybir
        from concourse import bass2jax

        nc = _get_compiled()
        bass2jax.install_neuronx_cc_hook()

        partition_name = (nc.partition_id_tensor.name
                          if nc.partition_id_tensor else None)
        in_names = []
        out_names = []
        out_avals = []
        for alloc in nc.m.functions[0].allocations:
            if not isinstance(alloc, mybir.MemoryLocationSet):
                continue
            name = alloc.memorylocations[0].name
            if alloc.kind == "ExternalInput":
                if name != partition_name:
                    in_names.append(name)
            elif alloc.kind == "ExternalOutput":
                out_names.append(name)
                out_avals.append(jax.core.ShapedArray(
                    tuple(alloc.tensor_shape), mybir.dt.np(alloc.dtype)))
        n_outs = len(out_names)
        all_names = in_names + out_names
        if partition_name is not None:
            all_names = all_names + [partition_name]

        def _body(*args):
            operands = list(args)
            if partition_name is not None:
                operands.append(bass2jax.partition_id_tensor())
            outs = bass2jax._bass_exec_p.bind(
                *operands,
                out_avals=tuple(out_avals),
                in_names=tuple(all_names),
                out_names=tuple(out_names),
                lowering_input_output_aliases=(),
                sim_require_finite=True,
                sim_require_nnan=True,
                nc=nc,
            )
            return tuple(outs)

        devices = jax.devices()[:NCORES]
        mesh = Mesh(np.asarray(devices), ("core",))
        # xT is identical on every core: replicate it instead of concatenating
        # 8 copies on the host.
        in_specs = tuple(PartitionSpec() if name == "xT" else PartitionSpec("core")
                         for name in in_names)
        sharded = jax.jit(
            shard_map(_body, mesh=mesh,
                      in_specs=in_specs + (PartitionSpec("core"),) * n_outs,
                      out_specs=(PartitionSpec("core"),) * n_outs,
                      check_rep=False),
            keep_unused=True)

        # separate jit: on-device sum of the 8 per-core partials (all-reduce)
        def _reduce(a):
            return jnp.sum(a.reshape((NCORES,) + tuple(out_avals[0].shape))
                           .astype(jnp.float32), axis=0)
        reduce_jit = jax.jit(_reduce)

        out_shapes = [tuple(a.shape) for a in out_avals]
        out_dtypes = [a.dtype for a in out_avals]
        zeros_dev = [None]

        from jax.sharding import NamedSharding
        rep_shd = NamedSharding(mesh, PartitionSpec())

        def call(in_maps):
            args = []
            for name in in_names:
                if name == "xT":
                    # one host->device transfer, then device-side broadcast
                    xd = jax.device_put(np.asarray(in_maps[0][name]),
                                        devices[0])
                    args.append(jax.device_put(xd, rep_shd))
                else:
                    args.append(np.concatenate(
                        [np.asarray(m[name]) for m in in_maps], axis=0))
            if zeros_dev[0] is None:
                from jax.sharding import NamedSharding
                shd = NamedSharding(mesh, PartitionSpec("core"))
                zeros_dev[0] = [
                    jax.device_put(
                        np.zeros((NCORES * sh[0],) + sh[1:], dt), shd)
                    for sh, dt in zip(out_shapes, out_dtypes)]
            outs = sharded(*args, *zeros_dev[0])
            try:
                summed = np.asarray(reduce_jit(outs[0]))
            except Exception:
                # device reduce unavailable: fetch partials, sum on host
                a = np.asarray(outs[0]).astype(np.float64)
                summed = a.reshape((NCORES,) + tuple(out_avals[0].shape)).sum(0)
            return {out_names[0]: summed}

        _RUNNER_STATE.update(sharded=sharded, in_names=in_names,
                             out_shapes=out_shapes, out_dtypes=out_dtypes,
                             call=call, mesh=mesh)
        _RUNNER = call
    return _RUNNER


def run(x, Wq, Wk, Wv, Wo, bo, trace=False):
    from concourse._compat import axon_active
    in_maps = make_in_maps(x, Wq, Wk, Wv, Wo)
    if axon_active():
        summed = _get_runner()(in_maps)
        acc = summed["outT"].astype(np.float64)
        results = summed
    else:
        # native /dev/neuron* path (non-axon environments)
        from concourse import bass_utils
        r = bass_utils.run_bass_kernel_spmd(
            _get_compiled(), in_maps, core_ids=list(range(NCORES)), trace=trace)
        results = r.results
        acc = np.zeros((B, D, S), dtype=np.float64)
        for c in range(NCORES):
            acc += np.asarray(results[c]["outT"], dtype=np.float64)
    out = acc.transpose(0, 2, 1) + np.asarray(bo, dtype=np.float64)
    return out.astype(np.float32), results


def kernel(x, Wq, Wk, Wv, Wo, bo):
    out, _ = run(np.asarray(x), np.asarray(Wq), np.asarray(Wk),
                 np.asarray(Wv), np.asarray(Wo), np.asarray(bo))
    return out


# revision 67
# speedup vs baseline: 1.4269x; 1.0010x over previous
"""Multi-head attention (B=2, S=2048, D=1024, H=16, no mask) on 8 TRN2 cores.

Sharding: tensor-parallel over heads — 2 heads per core. Each core computes
its heads' QKV projections, attention, and a partial out-projection
(row-sharded Wo); the host sums the 8 partials and adds the bias (the
all-reduce happens at gather time).

Device layout (per core), v3 — bf16 dataflow, software-pipelined attention,
transposed PV with per-partition softmax normalization:
  - All tensor data bf16 (x, W, q/k/v, exp-scores, ctx, partial out);
    matmul accumulation and softmax statistics stay fp32 in PSUM.
  - qT/kT kept transposed (feat-on-partitions); v projected DIRECTLY in
    token-major layout ([tokens, head, HD+1] va tiles, ones column
    prebaked) via per-token-tile matmuls — no PE transpose pass for v.
  - scoresT[k, q] = k @ qT per (head, key-tile): the two heads' K=64
    matmuls sit on array row-groups 0/64 and run concurrently on HW.
  - The attention inner loop is software-pipelined: QK(sk+1) is emitted
    BEFORE PV(sk), so the in-order PE never parks the next score matmul
    behind a PV that waits on exp(sk); ACT (the exp engine, the largest
    single engine load) stays saturated.
  - PV is TRANSPOSED: lhsT = exp-scores slice [keys 128, q-tile 128]
    (stationary, full 128-wide array), rhs = va [keys, HD+1] -> psum
    ctx[q, HD+1]; the va ones column puts the softmax denominator in psum
    COLUMN 64, i.e. a per-partition scalar.
  - Normalize: DVE reciprocal on the [128, 1] denominator column +
    per-partition tensor_scalar multiply (no cross-partition scatter
    DMAs, no PE broadcast matmuls), then PE transposes per (head,
    q-tile) put ctx back feature-major for the out-projection. Transpose
    outputs must start at psum partition 0, so head 1's rows reach ctxT
    partitions 64..127 via an SBUF-SBUF DMA — except in the drain tail,
    where the out-projection instead runs as two K=64 accumulating
    matmuls (head 1 read from its partition-0 staging tile) to keep that
    DMA off the critical path.
  - Cross-phase overlap via two 'pending unit' queues (PE-light normalize
    tails vs PE-heavy projection/out-projection blocks), drained one of
    each per sk-step into the ACT-paced attention loop's PE bubbles; the
    next query block's first QK is pre-emitted inside the current block's
    last step so exp never waits at block boundaries.
"""
import numpy as np

B = 2
S = 2048
D = 1024
H = 16
HD = 64
NCORES = 8
HPC = H // NCORES       # heads per core
FPC = HPC * HD          # 128 features per core


def build_mha_kernel(tc, outT, xT, wqT, wkT, wvT, woT, s=S, d=D):
    """Emit the per-core MHA program.

    outT: [B, d, s] bf16 (partial output, transposed, per-batch)
    xT:   [B, d, s] bf16
    wqT/wkT/wvT: [128, d//128, FPC] bf16, host pre-arranged so the
        weight DMA is contiguous (wqT pre-scaled by 1/sqrt(HD))
    woT:  [FPC, d] bf16
    """
    import concourse.mybir as mybir
    from contextlib import ExitStack

    nc = tc.nc
    f32 = mybir.dt.float32
    f32r = mybir.dt.float32r
    bf16 = mybir.dt.bfloat16
    Exp = mybir.ActivationFunctionType.Exp

    KT = d // 128           # contraction tiles for projections
    SK = s // 128           # key tiles per batch
    NCH = s // 512          # 512-token chunks
    NQB = s // 512          # query blocks
    QTPB = 4                # 128-wide q-tiles per 512-wide query block

    with ExitStack() as es:
        consts = es.enter_context(tc.tile_pool(name="consts", bufs=1))
        wpool = es.enter_context(tc.tile_pool(name="w", bufs=1))
        xpool = es.enter_context(tc.tile_pool(name="xt", bufs=2))
        qkv = es.enter_context(tc.tile_pool(name="qkv", bufs=2))
        vapool = es.enter_context(tc.tile_pool(name="va", bufs=1))
        epool = es.enter_context(tc.tile_pool(name="exp", bufs=2))
        cpool = es.enter_context(tc.tile_pool(name="ctxT", bufs=2))
        spool = es.enter_context(tc.tile_pool(name="small", bufs=2))
        opool = es.enter_context(tc.tile_pool(name="o", bufs=4))
        ps_sc = es.enter_context(tc.tile_pool(name="pssc", bufs=2, space="PSUM"))
        ps_ctx = es.enter_context(tc.tile_pool(name="psctx", bufs=1, space="PSUM"))
        ps_wk = es.enter_context(tc.tile_pool(name="pswk", bufs=2, space="PSUM"))

        from concourse.masks import make_identity
        identity = consts.tile([128, 128], bf16, tag="ident")
        make_identity(nc, identity[:])

        # --- weights (resident whole kernel). wq+wk on the SP queue (needed
        # first); wv/wo on the Pool queue. The ACT queue is kept clear of
        # DMA dispatches so exp issue is never delayed.
        wk_sb = wpool.tile([128, KT, FPC], bf16, tag="wk")
        nc.sync.dma_start(wk_sb[:], wkT)
        wq_sb = wpool.tile([128, KT, FPC], bf16, tag="wq")
        wv_sb = wpool.tile([128, KT, FPC], bf16, tag="wv")
        nc.gpsimd.dma_start(wv_sb[:], wvT)
        wo_sb = wpool.tile([128, d], bf16, tag="wo")
        nc.gpsimd.dma_start(wo_sb[:], woT)

        # --- va tiles: [tokens 128, head, HD+1] with a persistent ones
        # column at [:, :, HD] (written once; v columns rewritten per batch)
        vas = []
        for sk in range(SK):
            va = vapool.tile([128, HPC, HD + 1], bf16, tag=f"va{sk}")
            nc.gpsimd.memset(va[:, :, HD:HD + 1], 1.0)
            vas.append(va)

        # --- x loads. batch 0: per-(tile, chunk) pieces, chunk-major, so the
        # first k-proj chunk can start after ~1/4 of the data; batch 1: whole
        # tiles. Split across the SP and Pool HWDGE queues.
        xts = {}
        for b in range(B):
            for k in range(KT):
                xts[(b, k)] = xpool.tile([128, s], bf16, tag=f"x{k}",
                                         name=f"x{b}_{k}")
        for ch in range(NCH):
            for k in range(KT):
                eng = nc.gpsimd if (ch * KT + k) % 2 else nc.sync
                cs = slice(ch * 512, (ch + 1) * 512)
                eng.dma_start(xts[(0, k)][:, cs], xT[0, k * 128:(k + 1) * 128, cs])
            if ch == 0:
                # wq sits behind the chunk-0 x pieces: k-proj's critical
                # path is not delayed, and wq still lands before q0-proj
                nc.sync.dma_start(wq_sb[:], wqT)
        for k in range(KT):
            eng = nc.gpsimd if k % 2 else nc.sync
            eng.dma_start(xts[(1, k)][:], xT[1, k * 128:(k + 1) * 128, :])

        # ---------- unit builders (each emits a small instruction group) ----
        def proj_chunk(dst, w_sb, b, ch):
            # feat-major projection chunk: dst[:, ch*512:+512] (for q/k)
            cs = slice(ch * 512, (ch + 1) * 512)
            pt = ps_wk.tile([128, 512], f32, tag="wk")
            for k in range(KT):
                nc.tensor.matmul(pt[:], w_sb[:, k, :], xts[(b, k)][:, cs],
                                 start=(k == 0), stop=(k == KT - 1))
            nc.vector.tensor_copy(dst[:, cs], pt[:])

        def proj_chunk_halves(dst, w_sb, b, ch):
            # proj_chunk split into two pending units (halves the PE burst a
            # unit injects into the ACT-paced loop). The psum ring has 2
            # slots and at most one other unit runs between the halves, so
            # the accumulator survives; the two halves MUST stay adjacent
            # in the heavies queue.
            cs = slice(ch * 512, (ch + 1) * 512)
            state = {}

            def half1():
                pt = ps_wk.tile([128, 512], f32, tag="wk", name="pt")
                for k in range(KT // 2):
                    nc.tensor.matmul(pt[:], w_sb[:, k, :], xts[(b, k)][:, cs],
                                     start=(k == 0), stop=False)
                state["pt"] = pt

            def half2():
                pt = state.pop("pt")
                for k in range(KT // 2, KT):
                    nc.tensor.matmul(pt[:], w_sb[:, k, :], xts[(b, k)][:, cs],
                                     start=False, stop=(k == KT - 1))
                nc.vector.tensor_copy(dst[:, cs], pt[:])

            return [half1, half2]

        def vproj_unit(b, sk):
            # token-major v projection: va[sk] tokens sk*128..+128, both heads
            ts_ = slice(sk * 128, (sk + 1) * 128)
            vp = ps_wk.tile([128, 512], f32, tag="wk")
            for k in range(KT):
                nc.tensor.matmul(vp[:, 0:FPC], xts[(b, k)][:, ts_],
                                 wv_sb[:, k, :],
                                 start=(k == 0), stop=(k == KT - 1))
            src = vp[:, 0:FPC].rearrange("p (j f) -> p j f", j=HPC)
            nc.vector.tensor_copy(vas[sk][:, :, 0:HD], src)

        def emit_outproj(ctxT_b, bb, m, ch, eng, split=False):
            # one partial out-projection unit: outT[bb] tile (m, ch)
            ms = slice(m * 128, (m + 1) * 128)
            cs = slice(ch * 512, (ch + 1) * 512)
            op = ps_wk.tile([128, 512], f32, tag="wk")
            if split:
                # tail variant: head 1's context read from its SBUF staging
                # tile (partitions 0..63) via a second K=64 matmul
                nc.tensor.matmul(op[:], wo_sb[0:HD, ms], ctxT_b[0:HD, cs],
                                 start=True, stop=False)
                nc.tensor.matmul(op[:], wo_hi[:, ms], cn2_stash[ch][:],
                                 start=False, stop=True)
            else:
                nc.tensor.matmul(op[:], wo_sb[:, ms], ctxT_b[:, cs],
                                 start=True, stop=True)
            ot = opool.tile([128, 512], bf16, tag="ot")
            if eng is nc.scalar:
                eng.copy(ot[:], op[:])
            else:
                eng.tensor_copy(ot[:], op[:])
            nc.sync.dma_start(outT[bb, ms, cs], ot[:])

        # pending unit queues, drained into the attention loop's PE bubbles:
        # `lights` are PE-light normalize tails, `heavies` are PE-heavy
        # projection / out-projection blocks. One of each per sk-step.
        lights = []
        heavies = []

        def drain(n_heavy=1):
            if lights:
                lights.pop(0)()
            popped = 0
            while heavies and popped < n_heavy:
                heavies.pop(0)()
                popped += 1
            if not popped and lights:
                lights.pop(0)()

        # q/k tiles per batch: allocate both batches' ring slots up front so
        # units queued during batch 0 write the buffers batch 1 will read.
        qTs = [qkv.tile([128, s], bf16, tag="q", name=f"qT{b}")
               for b in range(B)]
        kTs = [qkv.tile([128, s], bf16, tag="k", name=f"kT{b}")
               for b in range(B)]
        ctxTs = [cpool.tile([128, s], bf16, tag="ctxT", name=f"ctxT{b}")
                 for b in range(B)]

        # second copy of wo's high rows at base partition 0: lets the drain
        # tail's out-projections take head 1's context from SBUF directly
        # (two K=64 accumulating matmuls) instead of waiting on the
        # cross-partition DMA into ctxT
        wo_hi = wpool.tile([HD, d], bf16, tag="wo_hi")
        nc.vector.tensor_copy(wo_hi[:], wo_sb[HD:2 * HD, :])
        cn2_stash = {}

        def norm_unit(h, cs_h, ctxT, qb, release, skip_dma=False, tail=False):
            # per (head, qb): 1/denom columns for all 4 q-tiles in one DVE
            # reciprocal, per-partition multiplies on Pool (SBUF-only), PE
            # transposes back to feature-major into ONE psum tile (head 1
            # lands on partitions 64..127 via col tile_position), single
            # DVE evacuation into ctxT.
            rc = spool.tile([128, QTPB], f32, tag=f"rc{h}", name="rc")
            nc.vector.reciprocal(rc[:], cs_h[:, :, HD])
            cn = spool.tile([128, QTPB, HD], bf16, tag=f"cn{h}", name="cn")
            # the drain tail has an idle DVE: run head 0's multiplies there
            # so the two heads' normalize chains overlap
            mul_eng = nc.vector if (tail and h == 0) else nc.gpsimd
            with nc.allow_low_precision(reason="ctx rounds to bf16 anyway"):
                for qt in range(QTPB):
                    mul_eng.tensor_scalar_mul(cn[:, qt, :],
                                              cs_h[:, qt, 0:HD],
                                              rc[:, qt:qt + 1])
            # bf16 transposes at 1 cycle/row (vs 2 for f32) into a packed
            # bf16 view of an f32 psum work tile
            wkt = ps_wk.tile([128, 512], f32, tag="wk", name="tpw")
            tpv = wkt[0:HD, 0:256].bitcast(bf16)
            for qt in range(QTPB):
                nc.tensor.transpose(tpv[:, qt * 128:(qt + 1) * 128],
                                    cn[:, qt, :], identity[:])
            if h == 0:
                nc.vector.tensor_copy(ctxT[0:HD, qb * 512:(qb + 1) * 512],
                                      tpv)
            else:
                # transpose outputs must start at psum partition 0; head 1's
                # rows reach ctxT partitions 64..127 via an SBUF-SBUF DMA
                cn2 = spool.tile([HD, 512], bf16, tag="cn2", name="cn2")
                nc.vector.tensor_copy(cn2[:], tpv)
                if skip_dma:
                    cn2_stash[qb] = cn2
                else:
                    nc.gpsimd.dma_start(
                        ctxT[HD:2 * HD, qb * 512:(qb + 1) * 512], cn2[:])
            if release:
                # ctxT columns for this qb are complete: release the
                # out-projection units for it
                heavies.extend(release)

        # batch 0 projections: attention can start once k-chunk0 (keys 0-511
        # = sk 0-3) and q-chunk0 are in; later k chunks are woven ahead of
        # the v-proj units into qb0's early steps (chunk c is consumed from
        # sk=4c, drained at step c-1). Steps 0..NCH-2 drain two heavies so
        # the v-proj units stay on their just-in-time schedule.
        proj_chunk(kTs[0], wk_sb, 0, 0)
        proj_chunk(qTs[0], wq_sb, 0, 0)
        kh = []
        for ch in range(1, NCH):
            kh += proj_chunk_halves(kTs[0], wk_sb, 0, ch)
        vs = [(lambda sk=sk: vproj_unit(0, sk)) for sk in range(SK)]
        if NCH == 4:
            # interleave the 6 k-halves so each v-proj unit still drains at
            # its just-in-time step (see drain(2) schedule below); each
            # half-pair stays 2 apart (one ring allocation between halves)
            order = [vs[0], kh[0], vs[1], kh[1], vs[2], vs[3], kh[2], vs[4],
                     kh[3], vs[5], vs[6], vs[7], kh[4], vs[8], kh[5], vs[9]]
            order += vs[10:]
        else:
            order = list(vs)
            for i, u in enumerate(kh):
                order.insert(2 * i + 1, u)
        heavies.extend(order)

        # ---------- attention ----------
        b1proj_left = [0]
        qk_pre = {}
        for b in range(B):
            qT, kT, ctxT = qTs[b], kTs[b], ctxTs[b]
            for qb in range(NQB):
                qs = slice(qb * 512, (qb + 1) * 512)

                # ctx accumulators: per head, 4 q-tiles x (HD+1) packed in
                # one PSUM bank ([128, 4, 128] fp32, slices [:, qt, 0:65])
                accs = [ps_ctx.tile([128, QTPB, 128], f32, tag=f"acc{h}",
                                    name=f"acc{h}")
                        for h in range(HPC)]
                ets = {}

                def emit_qk(sk, qT=qT, kT=kT, qs=qs, ets=ets, store=None):
                    sps = ps_sc.tile([128, 2 * 512], f32, tag="sc", name="sps")
                    for h in range(HPC):
                        hr = slice(h * HD, (h + 1) * HD)
                        nc.tensor.matmul(
                            sps[:, h * 512:(h + 1) * 512],
                            kT[hr, sk * 128:(sk + 1) * 128],
                            qT[hr, qs], start=True, stop=True)
                    et = epool.tile([128, 2 * 512], bf16, tag=f"et{sk % 2}",
                                    name="et")
                    nc.scalar.activation(et[:], sps[:], Exp)
                    if store is not None:
                        qk_pre[store] = et
                    else:
                        ets[sk] = et

                def emit_pv(sk, accs=accs, ets=ets):
                    # transposed PV: exp-scores stationary (full width),
                    # va moving; ctx[q, hd] + denominator column in psum
                    et = ets.pop(sk)
                    for h in range(HPC):
                        for qt in range(QTPB):
                            # the 4 q-tile accumulators share one psum bank
                            # (= one 2KB zero region): only the bank's FIRST
                            # matmul starts (zeroing the whole region), only
                            # its LAST stops
                            nc.tensor.matmul(
                                accs[h][:, qt, 0:HD + 1],
                                et[:, h * 512 + qt * 128:
                                   h * 512 + (qt + 1) * 128],
                                vas[sk][:, h, :],
                                start=(sk == 0 and qt == 0),
                                stop=(sk == SK - 1 and qt == QTPB - 1))

                # software-pipelined sk loop: QK(sk+1) lands before PV(sk),
                # pending units fill the gap where PV waits on exp. The
                # FIRST QK of this qb may have been pre-emitted in the
                # previous qb's last step (qk_pre holds its exp tile).
                if (b, qb) in qk_pre:
                    ets[0] = qk_pre.pop((b, qb))
                else:
                    emit_qk(0)
                if b == 0 and qb + 1 < NQB:
                    # next q chunk, as two PE-light front-of-queue units
                    # (not needed until the next qb; emitting directly here
                    # would stall exp behind 1.7us of projection)
                    qh = proj_chunk_halves(qT, wq_sb, 0, qb + 1)
                    lights.insert(0, qh[0])
                    lights.insert(1, qh[1])
                # PV trails QK by TWO steps: PV(sk) lands at step sk+1, so
                # the first PV of a qb is emitted one step into its loop and
                # never stalls on the previous qb's accumulator evacuation.
                # The final qb reverts to trail-1 so its last PV (and the
                # whole drain tail) starts one step earlier.
                trail = 1
                for sk in range(SK):
                    if sk + 1 < SK:
                        emit_qk(sk + 1)
                    elif qb + 1 < NQB:
                        # pre-emit the next qb's first QK so its exp starts
                        # right after this qb's last one
                        emit_qk(0, qs=slice((qb + 1) * 512, (qb + 2) * 512),
                                store=(b, qb + 1))
                    elif b == 0 and b1proj_left[0] == 0:
                        # batch seam: batch 1's q/k are ready (full-size
                        # schedule) - pre-emit its first QK too
                        emit_qk(0, qT=qTs[1], kT=kTs[1],
                                qs=slice(0, 512), store=(1, 0))
                    drain(2 if (b == 0 and qb == 0 and
                                sk in (1, 2, 4, 5, 8, 9)) else 1)
                    if sk >= trail:
                        emit_pv(sk - trail)
                if trail:
                    emit_pv(SK - 1)

                # evacuate the raw accumulators to SBUF right away (frees
                # the psum banks for the next qb); normalize tails become
                # PE-light pending units.
                css = []
                for h in range(HPC):
                    cs_h = spool.tile([128, QTPB, HD + 1], f32, tag=f"cs{h}",
                                      name=f"cs{h}")
                    if b == B - 1 and qb == NQB - 1 and h == 0:
                        # drain tail: ACT is idle; run one of the two
                        # evacuations there so they don't serialize on DVE
                        nc.scalar.copy(cs_h[:], accs[h][:, :, 0:HD + 1])
                    else:
                        nc.vector.tensor_copy(cs_h[:], accs[h][:, :, 0:HD + 1])
                    css.append(cs_h)

                # out-projection units for this qb, released by the last
                # normalize unit (they read the ctxT columns it completes).
                # The final qb's units run in the drain tail where ACT is
                # idle: alternate their psum evacuations DVE/ACT there.
                tail_qb = (b == B - 1 and qb == NQB - 1)
                opr = [(lambda c=ctxT, bb=b, mm=m, cc=qb, sp=tail_qb,
                        e=(nc.scalar if (tail_qb and m % 2) else nc.vector):
                        emit_outproj(c, bb, mm, cc, e, split=sp))
                       for m in range(KT)]
                # head 1 first: its ctxT rows travel by SBUF-SBUF DMA, so
                # putting it ahead lets that latency overlap head 0's work
                for h in reversed(range(HPC)):
                    lights.append(
                        lambda h=h, cs_h=css[h], ctxT=ctxT, qb=qb,
                        release=(opr if h == 0 else None), sd=tail_qb:
                            norm_unit(h, cs_h, ctxT, qb, release,
                                      skip_dma=(sd and h == 1), tail=sd))

                if b == 0 and qb == min(1, NQB - 2):
                    # batch 1's k projection + all q chunks, into the tail
                    # of batch 0's attention (the qkv ring slots for batch 1
                    # are free; x(1) tiles have loaded long since)
                    def done(u):
                        def f():
                            u()
                            b1proj_left[0] -= 1
                        return f
                    units = []
                    for ch in range(NCH):
                        units += proj_chunk_halves(kTs[1], wk_sb, 1, ch)
                    for ch in range(NCH):
                        units += proj_chunk_halves(qTs[1], wq_sb, 1, ch)
                    b1proj_left[0] = len(units)
                    heavies.extend(done(u) for u in units)

            if b == 0:
                # make sure batch 1's q/k are in place before its attention
                # emits reads of them (no-op at full size: they drained
                # into qb2/qb3's bubbles already)
                while b1proj_left[0] > 0:
                    drain()
                # batch 1's v tiles refill inside batch 1's qb0 bubbles,
                # ahead of everything else queued (its PV needs va[sk] by
                # step sk)
                heavies[0:0] = [
                    (lambda sk=sk: vproj_unit(1, sk)) for sk in range(SK)]

        # drain everything left (batch-1 norm tails + outproj backlog)
        while lights or heavies:
            drain()


_CACHE = {}


def _get_compiled(s=S, d=D, reps=1):
    key = (s, d, reps)
    if key not in _CACHE:
        import concourse.bacc as bacc
        import concourse.tile as tile
        import concourse.mybir as mybir

        bf16 = mybir.dt.bfloat16
        nc = bacc.Bacc("TRN2", target_bir_lowering=False, debug=False)
        xT = nc.dram_tensor("xT", [B, d, s], bf16, kind="ExternalInput")
        wqT = nc.dram_tensor("wqT", [128, d // 128, FPC], bf16,
                             kind="ExternalInput")
        wkT = nc.dram_tensor("wkT", [128, d // 128, FPC], bf16,
                             kind="ExternalInput")
        wvT = nc.dram_tensor("wvT", [128, d // 128, FPC], bf16,
                             kind="ExternalInput")
        woT = nc.dram_tensor("woT", [FPC, d], bf16, kind="ExternalInput")
        outT = nc.dram_tensor("outT", [B, d, s], bf16, kind="ExternalOutput")
        with tile.TileContext(nc) as tc:
            for _ in range(reps):
                build_mha_kernel(tc, outT.ap(), xT.ap(), wqT.ap(), wkT.ap(),
                                 wvT.ap(), woT.ap(), s=s, d=d)
        nc.compile()
        _CACHE[key] = nc
    return _CACHE[key]


def _bf16(a):
    import ml_dtypes
    return np.ascontiguousarray(np.asarray(a, dtype=np.float32)).astype(
        ml_dtypes.bfloat16)


def make_in_maps(x, Wq, Wk, Wv, Wo):
    """Host-side shard prep: transpose x, slice + transpose weights per core."""
    b, s, d = x.shape
    xT = _bf16(x.transpose(0, 2, 1))
    scale = np.float32(1.0 / np.sqrt(HD))

    def prearr(wt):
        # [d, FPC] -> [128, d//128, FPC] so each SBUF partition row is one
        # contiguous DMA line (avoids 512B-descriptor strided reads)
        return _bf16(wt.reshape(d // 128, 128, FPC).transpose(1, 0, 2))

    in_maps = []
    for c in range(NCORES):
        if (c + 1) * FPC > d:
            # small-D sim configs: fewer head-slices than cores
            in_maps.append(in_maps[0])
            continue
        rs = slice(c * FPC, (c + 1) * FPC)
        in_maps.append({
            "xT": xT,
            "wqT": prearr((Wq[rs, :] * scale).T.astype(np.float32)),
            "wkT": prearr(Wk[rs, :].T.astype(np.float32)),
            "wvT": prearr(Wv[rs, :].T.astype(np.float32)),
            "woT": _bf16(Wo[:, rs].T),
        })
    return in_maps


_RUNNER = None
_RUNNER_STATE = {}


def _get_runner():
    """Build (once) a cached jitted SPMD executor mirroring
    bass2jax.run_bass_via_pjrt's multi-core path."""
    global _RUNNER
    if _RUNNER is None:
        import jax
        import jax.numpy as jnp
        from jax.sharding import Mesh, PartitionSpec
        from jax.experimental.shard_map import shard_map
        import concourse.mybir as mybir
        from concourse import bass2jax

        nc = _get_compiled()
        bass2jax.install_neuronx_cc_hook()

        partition_name = (nc.partition_id_tensor.name
                          if nc.partition_id_tensor else None)
        in_names = []
        out_names = []
        out_avals = []
        for alloc in nc.m.functions[0].allocations:
            if not isinstance(alloc, mybir.MemoryLocationSet):
                continue
            name = alloc.memorylocations[0].name
            if alloc.kind == "ExternalInput":
                if name != partition_name:
                    in_names.append(name)
            elif alloc.kind == "ExternalOutput":
                out_names.append(name)
                out_avals.append(jax.core.ShapedArray(
                    tuple(alloc.tensor_shape), mybir.dt.np(alloc.dtype)))
        n_outs = len(out_names)
        all_names = in_names + out_names
        if partition_name is not None:
            all_names = all_names + [partition_name]

        def _body(*args):
            operands = list(args)
            if partition_name is not None:
                operands.append(bass2jax.partition_id_tensor())
            outs = bass2jax._bass_exec_p.bind(
                *operands,
                out_avals=tuple(out_avals),
                in_names=tuple(all_names),
                out_names=tuple(out_names),
                lowering_input_output_aliases=(),
                sim_require_finite=True,
                sim_require_nnan=True,
                nc=nc,
            )
            return tuple(outs)

        devices = jax.devices()[:NCORES]
        mesh = Mesh(np.asarray(devices), ("core",))
        # xT is identical on every core: replicate it instead of concatenating
        # 8 copies on the host.
        in_specs = tuple(PartitionSpec() if name == "xT" else PartitionSpec("core")
                         for name in in_names)
        sharded = jax.jit(
            shard_map(_body, mesh=mesh,
                      in_specs=in_specs + (PartitionSpec("core"),) * n_outs,
                      out_specs=(PartitionSpec("core"),) * n_outs,
                      check_rep=False),
            keep_unused=True)

        # separate jit: on-device sum of the 8 per-core partials (all-reduce)
        def _reduce(a):
            return jnp.sum(a.reshape((NCORES,) + tuple(out_avals[0].shape))
                           .astype(jnp.float32), axis=0)
        reduce_jit = jax.jit(_reduce)

        out_shapes = [tuple(a.shape) for a in out_avals]
        out_dtypes = [a.dtype for a in out_avals]
        zeros_dev = [None]

        from jax.sharding import NamedSharding
        rep_shd = NamedSharding(mesh, PartitionSpec())

        def call(in_maps):
            args = []
            for name in in_names:
                if name == "xT":
                    # one host->device transfer, then device-side broadcast
                    xd = jax.device_put(np.asarray(in_maps[0][name]),
                                        devices[0])
                    args.append(jax.device_put(xd, rep_shd))
                else:
                    args.append(np.concatenate(
                        [np.asarray(m[name]) for m in in_maps], axis=0))
            if zeros_dev[0] is None:
                from jax.sharding import NamedSharding
                shd = NamedSharding(mesh, PartitionSpec("core"))
                zeros_dev[0] = [
                    jax.device_put(
                        np.zeros((NCORES * sh[0],) + sh[1:], dt), shd)
                    for sh, dt in zip(out_shapes, out_dtypes)]
            outs = sharded(*args, *zeros_dev[0])
            try:
                summed = np.asarray(reduce_jit(outs[0]))
            except Exception:
                # device reduce unavailable: fetch partials, sum on host
                a = np.asarray(outs[0]).astype(np.float64)
                summed = a.reshape((NCORES,) + tuple(out_avals[0].shape)).sum(0)
            return {out_names[0]: summed}

        _RUNNER_STATE.update(sharded=sharded, in_names=in_names,
                             out_shapes=out_shapes, out_dtypes=out_dtypes,
                             call=call, mesh=mesh)
        _RUNNER = call
    return _RUNNER


def run(x, Wq, Wk, Wv, Wo, bo, trace=False):
    from concourse._compat import axon_active
    in_maps = make_in_maps(x, Wq, Wk, Wv, Wo)
    if axon_active():
        summed = _get_runner()(in_maps)
        acc = summed["outT"].astype(np.float64)
        results = summed
    else:
        # native /dev/neuron* path (non-axon environments)
        from concourse import bass_utils
        r = bass_utils.run_bass_kernel_spmd(
            _get_compiled(), in_maps, core_ids=list(range(NCORES)), trace=trace)
        results = r.results
        acc = np.zeros((B, D, S), dtype=np.float64)
        for c in range(NCORES):
            acc += np.asarray(results[c]["outT"], dtype=np.float64)
    out = acc.transpose(0, 2, 1) + np.asarray(bo, dtype=np.float64)
    return out.astype(np.float32), results


def kernel(x, Wq, Wk, Wv, Wo, bo):
    out, _ = run(np.asarray(x), np.asarray(Wq), np.asarray(Wk),
                 np.asarray(Wv), np.asarray(Wo), np.asarray(bo))
    return out


# revision 71
# speedup vs baseline: 1.4280x; 1.0007x over previous
"""Multi-head attention (B=2, S=2048, D=1024, H=16, no mask) on 8 TRN2 cores.

Sharding: tensor-parallel over heads — 2 heads per core. Each core computes
its heads' QKV projections, attention, and a partial out-projection
(row-sharded Wo); the host sums the 8 partials and adds the bias (the
all-reduce happens at gather time).

Device layout (per core), v3 — bf16 dataflow, software-pipelined attention,
transposed PV with per-partition softmax normalization:
  - All tensor data bf16 (x, W, q/k/v, exp-scores, ctx, partial out);
    matmul accumulation and softmax statistics stay fp32 in PSUM.
  - qT/kT kept transposed (feat-on-partitions); v projected DIRECTLY in
    token-major layout ([tokens, head, HD+1] va tiles, ones column
    prebaked) via per-token-tile matmuls — no PE transpose pass for v.
  - scoresT[k, q] = k @ qT per (head, key-tile): the two heads' K=64
    matmuls sit on array row-groups 0/64 and run concurrently on HW.
  - The attention inner loop is software-pipelined: QK(sk+1) is emitted
    BEFORE PV(sk), so the in-order PE never parks the next score matmul
    behind a PV that waits on exp(sk); ACT (the exp engine, the largest
    single engine load) stays saturated.
  - PV is TRANSPOSED: lhsT = exp-scores slice [keys 128, q-tile 128]
    (stationary, full 128-wide array), rhs = va [keys, HD+1] -> psum
    ctx[q, HD+1]; the va ones column puts the softmax denominator in psum
    COLUMN 64, i.e. a per-partition scalar.
  - Normalize: DVE reciprocal on the [128, 1] denominator column +
    per-partition tensor_scalar multiply (no cross-partition scatter
    DMAs, no PE broadcast matmuls), then PE transposes per (head,
    q-tile) put ctx back feature-major for the out-projection. Transpose
    outputs must start at psum partition 0, so head 1's rows reach ctxT
    partitions 64..127 via an SBUF-SBUF DMA — except in the drain tail,
    where the out-projection instead runs as two K=64 accumulating
    matmuls (head 1 read from its partition-0 staging tile) to keep that
    DMA off the critical path.
  - Cross-phase overlap via two 'pending unit' queues (PE-light normalize
    tails vs PE-heavy projection/out-projection blocks), drained one of
    each per sk-step into the ACT-paced attention loop's PE bubbles; the
    next query block's first QK is pre-emitted inside the current block's
    last step so exp never waits at block boundaries.
"""
import numpy as np

B = 2
S = 2048
D = 1024
H = 16
HD = 64
NCORES = 8
HPC = H // NCORES       # heads per core
FPC = HPC * HD          # 128 features per core


def build_mha_kernel(tc, outT, xT, wqT, wkT, wvT, woT, s=S, d=D):
    """Emit the per-core MHA program.

    outT: [B, d, s] bf16 (partial output, transposed, per-batch)
    xT:   [B, d, s] bf16
    wqT/wkT/wvT: [128, d//128, FPC] bf16, host pre-arranged so the
        weight DMA is contiguous (wqT pre-scaled by 1/sqrt(HD))
    woT:  [FPC, d] bf16
    """
    import concourse.mybir as mybir
    from contextlib import ExitStack

    nc = tc.nc
    f32 = mybir.dt.float32
    f32r = mybir.dt.float32r
    bf16 = mybir.dt.bfloat16
    Exp = mybir.ActivationFunctionType.Exp

    KT = d // 128           # contraction tiles for projections
    SK = s // 128           # key tiles per batch
    NCH = s // 512          # 512-token chunks
    NQB = s // 512          # query blocks
    QTPB = 4                # 128-wide q-tiles per 512-wide query block

    with ExitStack() as es:
        consts = es.enter_context(tc.tile_pool(name="consts", bufs=1))
        wpool = es.enter_context(tc.tile_pool(name="w", bufs=1))
        xpool = es.enter_context(tc.tile_pool(name="xt", bufs=2))
        qkv = es.enter_context(tc.tile_pool(name="qkv", bufs=2))
        vapool = es.enter_context(tc.tile_pool(name="va", bufs=1))
        epool = es.enter_context(tc.tile_pool(name="exp", bufs=2))
        cpool = es.enter_context(tc.tile_pool(name="ctxT", bufs=2))
        spool = es.enter_context(tc.tile_pool(name="small", bufs=2))
        opool = es.enter_context(tc.tile_pool(name="o", bufs=4))
        ps_sc = es.enter_context(tc.tile_pool(name="pssc", bufs=2, space="PSUM"))
        ps_ctx = es.enter_context(tc.tile_pool(name="psctx", bufs=1, space="PSUM"))
        ps_wk = es.enter_context(tc.tile_pool(name="pswk", bufs=2, space="PSUM"))

        from concourse.masks import make_identity
        identity = consts.tile([128, 128], bf16, tag="ident")
        make_identity(nc, identity[:])

        # --- weights (resident whole kernel). wq+wk on the SP queue (needed
        # first); wv/wo on the Pool queue. The ACT queue is kept clear of
        # DMA dispatches so exp issue is never delayed.
        wk_sb = wpool.tile([128, KT, FPC], bf16, tag="wk")
        nc.sync.dma_start(wk_sb[:], wkT)
        wq_sb = wpool.tile([128, KT, FPC], bf16, tag="wq")
        wv_sb = wpool.tile([128, KT, FPC], bf16, tag="wv")
        nc.gpsimd.dma_start(wv_sb[:], wvT)
        wo_sb = wpool.tile([128, d], bf16, tag="wo")
        nc.gpsimd.dma_start(wo_sb[:], woT)

        # --- va tiles: [tokens 128, head, HD+1] with a persistent ones
        # column at [:, :, HD] (written once; v columns rewritten per batch)
        vas = []
        for sk in range(SK):
            va = vapool.tile([128, HPC, HD + 1], bf16, tag=f"va{sk}")
            nc.gpsimd.memset(va[:, :, HD:HD + 1], 1.0)
            vas.append(va)

        # --- x loads. batch 0: per-(tile, chunk) pieces, chunk-major, so the
        # first k-proj chunk can start after ~1/4 of the data; batch 1: whole
        # tiles. Split across the SP and Pool HWDGE queues.
        xts = {}
        for b in range(B):
            for k in range(KT):
                xts[(b, k)] = xpool.tile([128, s], bf16, tag=f"x{k}",
                                         name=f"x{b}_{k}")
        for ch in range(NCH):
            for k in range(KT):
                eng = nc.gpsimd if (ch * KT + k) % 2 else nc.sync
                cs = slice(ch * 512, (ch + 1) * 512)
                eng.dma_start(xts[(0, k)][:, cs], xT[0, k * 128:(k + 1) * 128, cs])
            if ch == 0:
                # wq sits behind the chunk-0 x pieces: k-proj's critical
                # path is not delayed, and wq still lands before q0-proj
                nc.sync.dma_start(wq_sb[:], wqT)
        for k in range(KT):
            eng = nc.gpsimd if k % 2 else nc.sync
            eng.dma_start(xts[(1, k)][:], xT[1, k * 128:(k + 1) * 128, :])

        # ---------- unit builders (each emits a small instruction group) ----
        def proj_chunk(dst, w_sb, b, ch):
            # feat-major projection chunk: dst[:, ch*512:+512] (for q/k)
            cs = slice(ch * 512, (ch + 1) * 512)
            pt = ps_wk.tile([128, 512], f32, tag="wk")
            for k in range(KT):
                nc.tensor.matmul(pt[:], w_sb[:, k, :], xts[(b, k)][:, cs],
                                 start=(k == 0), stop=(k == KT - 1))
            nc.vector.tensor_copy(dst[:, cs], pt[:])

        def proj_chunk_halves(dst, w_sb, b, ch):
            # proj_chunk split into two pending units (halves the PE burst a
            # unit injects into the ACT-paced loop). The psum ring has 2
            # slots and at most one other unit runs between the halves, so
            # the accumulator survives; the two halves MUST stay adjacent
            # in the heavies queue.
            cs = slice(ch * 512, (ch + 1) * 512)
            state = {}

            def half1():
                pt = ps_wk.tile([128, 512], f32, tag="wk", name="pt")
                for k in range(KT // 2):
                    nc.tensor.matmul(pt[:], w_sb[:, k, :], xts[(b, k)][:, cs],
                                     start=(k == 0), stop=False)
                state["pt"] = pt

            def half2():
                pt = state.pop("pt")
                for k in range(KT // 2, KT):
                    nc.tensor.matmul(pt[:], w_sb[:, k, :], xts[(b, k)][:, cs],
                                     start=False, stop=(k == KT - 1))
                nc.vector.tensor_copy(dst[:, cs], pt[:])

            return [half1, half2]

        def vproj_unit(b, sk):
            # token-major v projection: va[sk] tokens sk*128..+128, both heads
            ts_ = slice(sk * 128, (sk + 1) * 128)
            vp = ps_wk.tile([128, 512], f32, tag="wk")
            for k in range(KT):
                nc.tensor.matmul(vp[:, 0:FPC], xts[(b, k)][:, ts_],
                                 wv_sb[:, k, :],
                                 start=(k == 0), stop=(k == KT - 1))
            src = vp[:, 0:FPC].rearrange("p (j f) -> p j f", j=HPC)
            nc.vector.tensor_copy(vas[sk][:, :, 0:HD], src)

        def emit_outproj(ctxT_b, bb, m, ch, eng, split=False):
            # one partial out-projection unit: outT[bb] tile (m, ch)
            ms = slice(m * 128, (m + 1) * 128)
            cs = slice(ch * 512, (ch + 1) * 512)
            op = ps_wk.tile([128, 512], f32, tag="wk")
            if split:
                # tail variant: head 1's context read from its SBUF staging
                # tile (partitions 0..63) via a second K=64 matmul
                nc.tensor.matmul(op[:], wo_sb[0:HD, ms], ctxT_b[0:HD, cs],
                                 start=True, stop=False)
                nc.tensor.matmul(op[:], wo_hi[:, ms], cn2_stash[ch][:],
                                 start=False, stop=True)
            else:
                nc.tensor.matmul(op[:], wo_sb[:, ms], ctxT_b[:, cs],
                                 start=True, stop=True)
            ot = opool.tile([128, 512], bf16, tag="ot")
            if eng is nc.scalar:
                eng.copy(ot[:], op[:])
            else:
                eng.tensor_copy(ot[:], op[:])
            nc.sync.dma_start(outT[bb, ms, cs], ot[:])

        # pending unit queues, drained into the attention loop's PE bubbles:
        # `lights` are PE-light normalize tails, `heavies` are PE-heavy
        # projection / out-projection blocks. One of each per sk-step.
        lights = []
        heavies = []

        def drain(n_heavy=1):
            if lights:
                lights.pop(0)()
            popped = 0
            while heavies and popped < n_heavy:
                heavies.pop(0)()
                popped += 1
            if not popped and lights:
                lights.pop(0)()

        # q/k tiles per batch: allocate both batches' ring slots up front so
        # units queued during batch 0 write the buffers batch 1 will read.
        qTs = [qkv.tile([128, s], bf16, tag="q", name=f"qT{b}")
               for b in range(B)]
        kTs = [qkv.tile([128, s], bf16, tag="k", name=f"kT{b}")
               for b in range(B)]
        ctxTs = [cpool.tile([128, s], bf16, tag="ctxT", name=f"ctxT{b}")
                 for b in range(B)]

        # second copy of wo's high rows at base partition 0: lets the drain
        # tail's out-projections take head 1's context from SBUF directly
        # (two K=64 accumulating matmuls) instead of waiting on the
        # cross-partition DMA into ctxT
        wo_hi = wpool.tile([HD, d], bf16, tag="wo_hi")
        nc.vector.tensor_copy(wo_hi[:], wo_sb[HD:2 * HD, :])
        cn2_stash = {}

        def norm_unit(h, cs_h, ctxT, qb, release, skip_dma=False, tail=False):
            # per (head, qb): 1/denom columns for all 4 q-tiles in one DVE
            # reciprocal, per-partition multiplies on Pool (SBUF-only), PE
            # transposes back to feature-major into ONE psum tile (head 1
            # lands on partitions 64..127 via col tile_position), single
            # DVE evacuation into ctxT.
            rc = spool.tile([128, QTPB], f32, tag=f"rc{h}", name="rc")
            nc.vector.reciprocal(rc[:], cs_h[:, :, HD])
            cn = spool.tile([128, QTPB, HD], bf16, tag=f"cn{h}", name="cn")
            # the drain tail has an idle DVE: run head 0's multiplies there
            # so the two heads' normalize chains overlap
            mul_eng = nc.vector if (tail and h == 0) else nc.gpsimd
            with nc.allow_low_precision(reason="ctx rounds to bf16 anyway"):
                for qt in range(QTPB):
                    mul_eng.tensor_scalar_mul(cn[:, qt, :],
                                              cs_h[:, qt, 0:HD],
                                              rc[:, qt:qt + 1])
            # bf16 transposes at 1 cycle/row (vs 2 for f32) into a packed
            # bf16 view of an f32 psum work tile
            wkt = ps_wk.tile([128, 512], f32, tag="wk", name="tpw")
            tpv = wkt[0:HD, 0:256].bitcast(bf16)
            for qt in range(QTPB):
                nc.tensor.transpose(tpv[:, qt * 128:(qt + 1) * 128],
                                    cn[:, qt, :], identity[:])
            if h == 0:
                nc.vector.tensor_copy(ctxT[0:HD, qb * 512:(qb + 1) * 512],
                                      tpv)
            else:
                # transpose outputs must start at psum partition 0; head 1's
                # rows reach ctxT partitions 64..127 via an SBUF-SBUF DMA
                cn2 = spool.tile([HD, 512], bf16, tag="cn2", name="cn2")
                nc.vector.tensor_copy(cn2[:], tpv)
                if skip_dma:
                    cn2_stash[qb] = cn2
                else:
                    nc.gpsimd.dma_start(
                        ctxT[HD:2 * HD, qb * 512:(qb + 1) * 512], cn2[:])
            if release:
                # ctxT columns for this qb are complete: release the
                # out-projection units for it
                heavies.extend(release)

        # batch 0 projections: QK(sk0) only needs keys 0..127, so the first
        # k-chunk is split — a quick N=128 pass gets attention started ~2.5us
        # earlier, and the N=384 remainder lands right after QK(0) inside
        # the SAME psum accumulation group (start zeroes the whole bank
        # once; the two passes write disjoint regions of it). Later k
        # chunks are woven ahead of the v-proj units into qb0's early
        # steps. Some steps drain two heavies so the v-proj units stay on
        # their just-in-time schedule.
        k0_state = {"todo": True}
        k0pt = ps_wk.tile([128, 512], f32, tag="wk", name="k0pt")
        for k in range(KT):
            nc.tensor.matmul(k0pt[:, 0:128], wk_sb[:, k, :],
                             xts[(0, k)][:, 0:128],
                             start=(k == 0), stop=(k == KT - 1))
        nc.vector.tensor_copy(kTs[0][:, 0:128], k0pt[:, 0:128])
        proj_chunk(qTs[0], wq_sb, 0, 0)
        kh = []
        for ch in range(1, NCH):
            kh += proj_chunk_halves(kTs[0], wk_sb, 0, ch)
        vs = [(lambda sk=sk: vproj_unit(0, sk)) for sk in range(SK)]
        if NCH == 4:
            # interleave the 6 k-halves so each v-proj unit still drains at
            # its just-in-time step (see drain(2) schedule below); each
            # half-pair stays 2 apart (one ring allocation between halves)
            order = [vs[0], kh[0], vs[1], kh[1], vs[2], vs[3], kh[2], vs[4],
                     kh[3], vs[5], vs[6], vs[7], kh[4], vs[8], kh[5], vs[9]]
            order += vs[10:]
        else:
            order = list(vs)
            for i, u in enumerate(kh):
                order.insert(2 * i + 1, u)
        heavies.extend(order)

        # ---------- attention ----------
        b1proj_left = [0]
        qk_pre = {}
        for b in range(B):
            qT, kT, ctxT = qTs[b], kTs[b], ctxTs[b]
            for qb in range(NQB):
                qs = slice(qb * 512, (qb + 1) * 512)

                # ctx accumulators: per head, 4 q-tiles x (HD+1) packed in
                # one PSUM bank ([128, 4, 128] fp32, slices [:, qt, 0:65])
                accs = [ps_ctx.tile([128, QTPB, 128], f32, tag=f"acc{h}",
                                    name=f"acc{h}")
                        for h in range(HPC)]
                ets = {}

                def emit_qk(sk, qT=qT, kT=kT, qs=qs, ets=ets, store=None):
                    sps = ps_sc.tile([128, 2 * 512], f32, tag="sc", name="sps")
                    for h in range(HPC):
                        hr = slice(h * HD, (h + 1) * HD)
                        nc.tensor.matmul(
                            sps[:, h * 512:(h + 1) * 512],
                            kT[hr, sk * 128:(sk + 1) * 128],
                            qT[hr, qs], start=True, stop=True)
                    et = epool.tile([128, 2 * 512], bf16, tag=f"et{sk % 2}",
                                    name="et")
                    nc.scalar.activation(et[:], sps[:], Exp)
                    if store is not None:
                        qk_pre[store] = et
                    else:
                        ets[sk] = et

                def emit_pv(sk, accs=accs, ets=ets):
                    # transposed PV: exp-scores stationary (full width),
                    # va moving; ctx[q, hd] + denominator column in psum
                    et = ets.pop(sk)
                    for h in range(HPC):
                        for qt in range(QTPB):
                            # the 4 q-tile accumulators share one psum bank
                            # (= one 2KB zero region): only the bank's FIRST
                            # matmul starts (zeroing the whole region), only
                            # its LAST stops
                            nc.tensor.matmul(
                                accs[h][:, qt, 0:HD + 1],
                                et[:, h * 512 + qt * 128:
                                   h * 512 + (qt + 1) * 128],
                                vas[sk][:, h, :],
                                start=(sk == 0 and qt == 0),
                                stop=(sk == SK - 1 and qt == QTPB - 1))

                # software-pipelined sk loop: QK(sk+1) lands before PV(sk),
                # pending units fill the gap where PV waits on exp. The
                # FIRST QK of this qb may have been pre-emitted in the
                # previous qb's last step (qk_pre holds its exp tile).
                if (b, qb) in qk_pre:
                    ets[0] = qk_pre.pop((b, qb))
                else:
                    emit_qk(0)
                if k0_state:
                    # finish kT[:, 128:512] (needed from QK(1)) behind QK(0),
                    # in its own ring tile (the ring's WAR tracking orders it
                    # after the first pass's evacuation)
                    k0_state.clear()
                    pt = ps_wk.tile([128, 512], f32, tag="wk", name="k0b")
                    for k in range(KT):
                        nc.tensor.matmul(pt[:, 0:384], wk_sb[:, k, :],
                                         xts[(0, k)][:, 128:512],
                                         start=(k == 0), stop=(k == KT - 1))
                    nc.vector.tensor_copy(kTs[0][:, 128:512], pt[:, 0:384])
                if b == 0 and qb + 1 < NQB:
                    # next q chunk, as two PE-light front-of-queue units
                    # (not needed until the next qb; emitting directly here
                    # would stall exp behind 1.7us of projection)
                    qh = proj_chunk_halves(qT, wq_sb, 0, qb + 1)
                    lights.insert(0, qh[0])
                    lights.insert(1, qh[1])
                # PV trails QK by TWO steps: PV(sk) lands at step sk+1, so
                # the first PV of a qb is emitted one step into its loop and
                # never stalls on the previous qb's accumulator evacuation.
                # The final qb reverts to trail-1 so its last PV (and the
                # whole drain tail) starts one step earlier.
                trail = 1
                for sk in range(SK):
                    if sk + 1 < SK:
                        emit_qk(sk + 1)
                    elif qb + 1 < NQB:
                        # pre-emit the next qb's first QK so its exp starts
                        # right after this qb's last one
                        emit_qk(0, qs=slice((qb + 1) * 512, (qb + 2) * 512),
                                store=(b, qb + 1))
                    elif b == 0 and b1proj_left[0] == 0:
                        # batch seam: batch 1's q/k are ready (full-size
                        # schedule) - pre-emit its first QK too
                        emit_qk(0, qT=qTs[1], kT=kTs[1],
                                qs=slice(0, 512), store=(1, 0))
                    drain(2 if (b == 0 and qb == 0 and
                                sk in (1, 2, 4, 5, 8, 9)) else 1)
                    if sk >= trail:
                        emit_pv(sk - trail)
                if trail:
                    emit_pv(SK - 1)

                # evacuate the raw accumulators to SBUF right away (frees
                # the psum banks for the next qb); normalize tails become
                # PE-light pending units.
                css = []
                for h in range(HPC):
                    cs_h = spool.tile([128, QTPB, HD + 1], f32, tag=f"cs{h}",
                                      name=f"cs{h}")
                    if b == B - 1 and qb == NQB - 1 and h == 0:
                        # drain tail: ACT is idle; run one of the two
                        # evacuations there so they don't serialize on DVE
                        nc.scalar.copy(cs_h[:], accs[h][:, :, 0:HD + 1])
                    else:
                        nc.vector.tensor_copy(cs_h[:], accs[h][:, :, 0:HD + 1])
                    css.append(cs_h)

                # out-projection units for this qb, released by the last
                # normalize unit (they read the ctxT columns it completes).
                # The final qb's units run in the drain tail where ACT is
                # idle: alternate their psum evacuations DVE/ACT there.
                tail_qb = (b == B - 1 and qb == NQB - 1)
                opr = [(lambda c=ctxT, bb=b, mm=m, cc=qb, sp=tail_qb,
                        e=(nc.scalar if (tail_qb and m % 2) else nc.vector):
                        emit_outproj(c, bb, mm, cc, e, split=sp))
                       for m in range(KT)]
                # head 1 first: its ctxT rows travel by SBUF-SBUF DMA, so
                # putting it ahead lets that latency overlap head 0's work
                for h in reversed(range(HPC)):
                    lights.append(
                        lambda h=h, cs_h=css[h], ctxT=ctxT, qb=qb,
                        release=(opr if h == 0 else None), sd=tail_qb:
                            norm_unit(h, cs_h, ctxT, qb, release,
                                      skip_dma=(sd and h == 1), tail=sd))

                if b == 0 and qb == min(1, NQB - 2):
                    # batch 1's k projection + all q chunks, into the tail
                    # of batch 0's attention (the qkv ring slots for batch 1
                    # are free; x(1) tiles have loaded long since)
                    def done(u):
                        def f():
                            u()
                            b1proj_left[0] -= 1
                        return f
                    units = []
                    for ch in range(NCH):
                        units += proj_chunk_halves(kTs[1], wk_sb, 1, ch)
                    for ch in range(NCH):
                        units += proj_chunk_halves(qTs[1], wq_sb, 1, ch)
                    b1proj_left[0] = len(units)
                    heavies.extend(done(u) for u in units)

            if b == 0:
                # make sure batch 1's q/k are in place before its attention
                # emits reads of them (no-op at full size: they drained
                # into qb2/qb3's bubbles already)
                while b1proj_left[0] > 0:
                    drain()
                # batch 1's v tiles refill inside batch 1's qb0 bubbles,
                # ahead of everything else queued (its PV needs va[sk] by
                # step sk)
                heavies[0:0] = [
                    (lambda sk=sk: vproj_unit(1, sk)) for sk in range(SK)]

        # drain everything left (batch-1 norm tails + outproj backlog)
        while lights or heavies:
            drain()


_CACHE = {}


def _get_compiled(s=S, d=D, reps=1):
    key = (s, d, reps)
    if key not in _CACHE:
        import concourse.bacc as bacc
        import concourse.tile as tile
        import concourse.mybir as mybir

        bf16 = mybir.dt.bfloat16
        nc = bacc.Bacc("TRN2", target_bir_lowering=False, debug=False)
        xT = nc.dram_tensor("xT", [B, d, s], bf16, kind="ExternalInput")
        wqT = nc.dram_tensor("wqT", [128, d // 128, FPC], bf16,
                             kind="ExternalInput")
        wkT = nc.dram_tensor("wkT", [128, d // 128, FPC], bf16,
                             kind="ExternalInput")
        wvT = nc.dram_tensor("wvT", [128, d // 128, FPC], bf16,
                             kind="ExternalInput")
        woT = nc.dram_tensor("woT", [FPC, d], bf16, kind="ExternalInput")
        outT = nc.dram_tensor("outT", [B, d, s], bf16, kind="ExternalOutput")
        with tile.TileContext(nc) as tc:
            for _ in range(reps):
                build_mha_kernel(tc, outT.ap(), xT.ap(), wqT.ap(), wkT.ap(),
                                 wvT.ap(), woT.ap(), s=s, d=d)
        nc.compile()
        _CACHE[key] = nc
    return _CACHE[key]


def _bf16(a):
    import ml_dtypes
    return np.ascontiguousarray(np.asarray(a, dtype=np.float32)).astype(
        ml_dtypes.bfloat16)


def make_in_maps(x, Wq, Wk, Wv, Wo):
    """Host-side shard prep: transpose x, slice + transpose weights per core."""
    b, s, d = x.shape
    xT = _bf16(x.transpose(0, 2, 1))
    scale = np.float32(1.0 / np.sqrt(HD))

    def prearr(wt):
        # [d, FPC] -> [128, d//128, FPC] so each SBUF partition row is one
        # contiguous DMA line (avoids 512B-descriptor strided reads)
        return _bf16(wt.reshape(d // 128, 128, FPC).transpose(1, 0, 2))

    in_maps = []
    for c in range(NCORES):
        if (c + 1) * FPC > d:
            # small-D sim configs: fewer head-slices than cores
            in_maps.append(in_maps[0])
            continue
        rs = slice(c * FPC, (c + 1) * FPC)
        in_maps.append({
            "xT": xT,
            "wqT": prearr((Wq[rs, :] * scale).T.astype(np.float32)),
            "wkT": prearr(Wk[rs, :].T.astype(np.float32)),
            "wvT": prearr(Wv[rs, :].T.astype(np.float32)),
            "woT": _bf16(Wo[:, rs].T),
        })
    return in_maps


_RUNNER = None
_RUNNER_STATE = {}


def _get_runner():
    """Build (once) a cached jitted SPMD executor mirroring
    bass2jax.run_bass_via_pjrt's multi-core path."""
    global _RUNNER
    if _RUNNER is None:
        import jax
        import jax.numpy as jnp
        from jax.sharding import Mesh, PartitionSpec
        from jax.experimental.shard_map import shard_map
        import concourse.mybir as mybir
        from concourse import bass2jax

        nc = _get_compiled()
        bass2jax.install_neuronx_cc_hook()

        partition_name = (nc.partition_id_tensor.name
                          if nc.partition_id_tensor else None)
        in_names = []
        out_names = []
        out_avals = []
        for alloc in nc.m.functions[0].allocations:
            if not isinstance(alloc, mybir.MemoryLocationSet):
                continue
            name = alloc.memorylocations[0].name
            if alloc.kind == "ExternalInput":
                if name != partition_name:
                    in_names.append(name)
            elif alloc.kind == "ExternalOutput":
                out_names.append(name)
                out_avals.append(jax.core.ShapedArray(
                    tuple(alloc.tensor_shape), mybir.dt.np(alloc.dtype)))
        n_outs = len(out_names)
        all_names = in_names + out_names
        if partition_name is not None:
            all_names = all_names + [partition_name]

        def _body(*args):
            operands = list(args)
            if partition_name is not None:
                operands.append(bass2jax.partition_id_tensor())
            outs = bass2jax._bass_exec_p.bind(
                *operands,
                out_avals=tuple(out_avals),
                in_names=tuple(all_names),
                out_names=tuple(out_names),
                lowering_input_output_aliases=(),
                sim_require_finite=True,
                sim_require_nnan=True,
                nc=nc,
            )
            return tuple(outs)

        devices = jax.devices()[:NCORES]
        mesh = Mesh(np.asarray(devices), ("core",))
        # xT is identical on every core: replicate it instead of concatenating
        # 8 copies on the host.
        in_specs = tuple(PartitionSpec() if name == "xT" else PartitionSpec("core")
                         for name in in_names)
        sharded = jax.jit(
            shard_map(_body, mesh=mesh,
                      in_specs=in_specs + (PartitionSpec("core"),) * n_outs,
                      out_specs=(PartitionSpec("core"),) * n_outs,
                      check_rep=False),
            keep_unused=True)

        # separate jit: on-device sum of the 8 per-core partials (all-reduce)
        def _reduce(a):
            return jnp.sum(a.reshape((NCORES,) + tuple(out_avals[0].shape))
                           .astype(jnp.float32), axis=0)
        reduce_jit = jax.jit(_reduce)

        out_shapes = [tuple(a.shape) for a in out_avals]
        out_dtypes = [a.dtype for a in out_avals]
        zeros_dev = [None]

        from jax.sharding import NamedSharding
        rep_shd = NamedSharding(mesh, PartitionSpec())

        def call(in_maps):
            args = []
            for name in in_names:
                if name == "xT":
                    # one host->device transfer, then device-side broadcast
                    xd = jax.device_put(np.asarray(in_maps[0][name]),
                                        devices[0])
                    args.append(jax.device_put(xd, rep_shd))
                else:
                    args.append(np.concatenate(
                        [np.asarray(m[name]) for m in in_maps], axis=0))
            if zeros_dev[0] is None:
                from jax.sharding import NamedSharding
                shd = NamedSharding(mesh, PartitionSpec("core"))
                zeros_dev[0] = [
                    jax.device_put(
                        np.zeros((NCORES * sh[0],) + sh[1:], dt), shd)
                    for sh, dt in zip(out_shapes, out_dtypes)]
            outs = sharded(*args, *zeros_dev[0])
            try:
                summed = np.asarray(reduce_jit(outs[0]))
            except Exception:
                # device reduce unavailable: fetch partials, sum on host
                a = np.asarray(outs[0]).astype(np.float64)
                summed = a.reshape((NCORES,) + tuple(out_avals[0].shape)).sum(0)
            return {out_names[0]: summed}

        _RUNNER_STATE.update(sharded=sharded, in_names=in_names,
                             out_shapes=out_shapes, out_dtypes=out_dtypes,
                             call=call, mesh=mesh)
        _RUNNER = call
    return _RUNNER


def run(x, Wq, Wk, Wv, Wo, bo, trace=False):
    from concourse._compat import axon_active
    in_maps = make_in_maps(x, Wq, Wk, Wv, Wo)
    if axon_active():
        summed = _get_runner()(in_maps)
        acc = summed["outT"].astype(np.float64)
        results = summed
    else:
        # native /dev/neuron* path (non-axon environments)
        from concourse import bass_utils
        r = bass_utils.run_bass_kernel_spmd(
            _get_compiled(), in_maps, core_ids=list(range(NCORES)), trace=trace)
        results = r.results
        acc = np.zeros((B, D, S), dtype=np.float64)
        for c in range(NCORES):
            acc += np.asarray(results[c]["outT"], dtype=np.float64)
    out = acc.transpose(0, 2, 1) + np.asarray(bo, dtype=np.float64)
    return out.astype(np.float32), results


def kernel(x, Wq, Wk, Wv, Wo, bo):
    out, _ = run(np.asarray(x), np.asarray(Wq), np.asarray(Wk),
                 np.asarray(Wv), np.asarray(Wo), np.asarray(bo))
    return out
